# revision 4
# baseline (speedup 1.0000x reference)
"""ChannelSA Trainium2 kernel: 8-way batch-parallel across NeuronCores.

kernel(**inputs) takes the FULL inputs (x [8,192,128,128], conv weights,
pos_emb) and returns the FULL output [8,192,128,128] fp32. Each core runs
an identical single-batch program (SPMD, no collectives).

Per-core pipeline:
  z   = W1 @ x                   1x1 conv, fp32r matmuls (K=192 -> 128+64)
  qkv = DW3x3(z)                 9 accumulating diagonal matmuls on TensorE
                                 over a zero-padded bf16 z layout (shifted APs)
  q,k -> bf16 DMA-transpose ->   per-head Gram banks [Gqk|Gqq|Gkk] in PSUM
  logits = Gqk / (|q||k| sqrt(48))   norms taken from the Gram diagonals;
                                 pos_emb is constant per softmax row: a no-op
  attn = softmax(logits)
  y = (W_out @ blockdiag(attn)) @ v   output projection fused with attn@v
"""
import math
from contextlib import ExitStack

import numpy as np

import concourse.bass as bass
import concourse.mybir as mybir
import concourse.tile as tile
from concourse.masks import make_identity

F32 = mybir.dt.float32
F32R = mybir.dt.float32r
BF16 = mybir.dt.bfloat16
AF = mybir.ActivationFunctionType

C = 192
CQKV = 576
H = 128
W = 128
L = H * W
HEADS = 4
DH = 48
R = 8                    # output image rows per chunk
NCHUNK = H // R
PADW = W + 2             # padded row stride in z tiles
ZROWS = R + 2            # rows held per z chunk (1 halo each side)
TAPS = [(di, dj) for di in (-1, 0, 1) for dj in (-1, 0, 1)]
BLKS = [(0, 128), (128, 256), (256, 384), (384, 512), (512, 576)]
N_CORES = 8

_MAX_DRAIN_WAITS = 1


def _patch_tail_drain():
    """The walrus in this image rejects >1 semaphore wait on the Tile tail
    drain instruction; split the waits across a chain of SP nops."""
    if getattr(tile.TileContext, "_drain_patched", False):
        return

    def _drain_and_barrier(self, tick_clock, wait_clock):
        from concourse.vector_clock import ScopedClock

        nc = self.nc
        drain_inst = nc.sync.drain()
        wait_clock.add_sem_waits(
            drain_inst.ins, ScopedClock({None: tick_clock.global_clock})
        )
        si = drain_inst.ins.sync_info
        waits = list(si.on_wait or [])
        if len(waits) > _MAX_DRAIN_WAITS:
            si.on_wait = waits[:_MAX_DRAIN_WAITS]
            rest = waits[_MAX_DRAIN_WAITS:]
            for i in range(0, len(rest), _MAX_DRAIN_WAITS):
                nop = nc.sync.nop(nofuse=True)
                nop.ins.sync_info = mybir.SyncInfo(
                    on_wait=rest[i : i + _MAX_DRAIN_WAITS], on_update=[]
                )
        nc.all_engine_barrier()
        assert self.sems is not None
        popped = nc._tile_sem_poison_stack.pop()
        assert popped is self._sem_poison
        nc.clear_and_free_semaphores(list(self.sems.allocated().values()))
        nc.all_engine_barrier()

    tile.TileContext._drain_and_barrier = _drain_and_barrier
    tile.TileContext._drain_patched = True


def build_nc(split_waits=True):
    _patch_tail_drain()
    nc = bass.Bass("TRN2", target_bir_lowering=False, debug=False)

    x_d = nc.declare_dram_parameter("x", [C, L], F32R, isOutput=False)
    w1t_d = nc.declare_dram_parameter("w1t", [C, CQKV], F32R, isOutput=False)
    wdw_d = nc.declare_dram_parameter("wdw", [CQKV, 9], F32, isOutput=False)
    woutt_d = nc.declare_dram_parameter("woutt", [C, C], F32, isOutput=False)
    y_d = nc.declare_dram_parameter("y", [C, L], F32, isOutput=True)

    with tile.TileContext(nc) as tc, ExitStack() as ctx:
        _body(ctx, tc, x_d, w1t_d, wdw_d, woutt_d, y_d)
    if split_waits:
        # CoreSim can't run the split module (its race detector wants sem
        # updates on every inst); the split is only needed for walrus.
        _split_excess_waits(nc)
    return nc


def _split_excess_waits(nc, maxw=1):
    """This walrus build accepts only one semaphore wait per instruction.
    Move excess waits onto same-engine no-ops inserted just before the
    offending instruction (same-engine program order preserves semantics)."""
    uid = [0]
    for f in nc.m.functions:
        for bb in f.blocks:
            il = bb.instructions
            out = []
            changed = False
            for inst in il:
                si = inst.sync_info
                waits = list(si.on_wait) if si and si.on_wait else []
                if len(waits) > maxw:
                    changed = True
                    rest, keep = waits[:-maxw], waits[-maxw:]
                    for i in range(0, len(rest), maxw):
                        uid[0] += 1
                        out.append(
                            mybir.InstNoOp(
                                name=f"I-waitsplit-{uid[0]}",
                                engine=inst.engine,
                                ins=[],
                                outs=[],
                                sync_info=mybir.SyncInfo(
                                    on_wait=rest[i : i + maxw], on_update=[]
                                ),
                            )
                        )
                    si.on_wait = keep
                out.append(inst)
            if changed:
                bb.instructions = out


def _body(ctx, tc, x_d, w1t_d, wdw_d, woutt_d, y_d):
    nc = tc.nc
    ncopy = [0]

    def copy(dst, src):
        # alternate PSUM->SBUF copies between ACT and DVE
        if ncopy[0] % 2 == 0:
            nc.scalar.copy(dst, src)
        else:
            nc.vector.tensor_copy(dst, src)
        ncopy[0] += 1

    const = ctx.enter_context(tc.tile_pool(name="const", bufs=1))
    persist = ctx.enter_context(tc.tile_pool(name="persist", bufs=1))

    # ---- constants / weights ----
    w1t0 = const.tile([128, CQKV], F32R, tag="w1t0")
    w1t1 = const.tile([64, CQKV], F32R, tag="w1t1")
    nc.sync.dma_start(w1t0[:], w1t_d[0:128, :])
    nc.sync.dma_start(w1t1[:], w1t_d[128:192, :])

    woutt0 = const.tile([128, C], F32, tag="woutt0")
    woutt1 = const.tile([64, C], F32, tag="woutt1")
    nc.sync.dma_start(woutt0[:], woutt_d[0:128, :])
    nc.sync.dma_start(woutt1[:], woutt_d[128:192, :])
    woutt0_bf = const.tile([128, C], BF16, tag="woutt0bf")
    woutt1_bf = const.tile([64, C], BF16, tag="woutt1bf")
    nc.vector.tensor_copy(woutt0_bf[:], woutt0[:])
    nc.vector.tensor_copy(woutt1_bf[:], woutt1[:])

    ident_bf = const.tile([128, 128], BF16, tag="identbf")
    make_identity(nc, ident_bf[:])
    ident48 = const.tile([48, 48], F32, tag="ident48")
    make_identity(nc, ident48[:])
    ones48 = const.tile([48, 1], F32, tag="ones48")
    nc.gpsimd.memset(ones48[:], 1.0)
    ones1x48 = const.tile([1, 48], F32, tag="ones1x48")
    nc.gpsimd.memset(ones1x48[:], 1.0)

    # dw weights -> 45 diagonal bf16 matrices
    diagw = []
    for b, (c0, c1) in enumerate(BLKS):
        p = c1 - c0
        wdw_sb = const.tile([p, 9], F32, tag=f"wdw{b}")
        nc.sync.dma_start(wdw_sb[:], wdw_d[c0:c1, :])
        row = []
        for t in range(9):
            dt_ = const.tile([p, p], BF16, tag=f"diag{b}_{t}")
            nc.vector.tensor_scalar_mul(dt_[:], ident_bf[0:p, 0:p], wdw_sb[:, t : t + 1])
            row.append(dt_)
        diagw.append(row)

    # ---- persistent state ----
    v0 = persist.tile([128, L], BF16, tag="v0")
    v1 = persist.tile([64, L], BF16, tag="v1")
    zt = [
        [
            persist.tile([c1 - c0, ZROWS, PADW], BF16, tag=f"z{s}_{b}", name=f"z{s}_{b}")
            for b, (c0, c1) in enumerate(BLKS)
        ]
        for s in range(2)
    ]
    for s in range(2):
        for b in range(5):
            nc.gpsimd.memset(zt[s][b][:], 0.0)

    ghs = persist.tile([48, HEADS * 144], F32, tag="ghs")

    # ---- phase A: chunked pipeline ----
    with (
        tc.tile_pool(name="gps", bufs=1, space="PSUM") as gps,
        tc.tile_pool(name="xp", bufs=2) as xp,
        tc.tile_pool(name="zps", bufs=2, space="PSUM") as zps,
        tc.tile_pool(name="qps", bufs=2, space="PSUM") as qps,
        tc.tile_pool(name="stp", bufs=2) as stp,
        tc.tile_pool(name="qktp", bufs=2) as qktp,
    ):
        # one PSUM bank per head: [Gqk | Gqq | Gkk] (one accumulation group each)
        gh = [gps.tile([48, 144], F32, tag=f"gh{h}", name=f"gh{h}") for h in range(HEADS)]
        for c in range(NCHUNK):
            zs = zt[c % 2]
            r0 = max(0, R * c - 1)
            r1 = min(H, R * c + R + 1)
            nrows = r1 - r0
            brow0 = r0 - (R * c - 1)  # buf row of image row r0

            xt0 = xp.tile([128, nrows, W], F32R, tag="x0")
            xt1 = xp.tile([64, nrows, W], F32R, tag="x1")
            nc.sync.dma_start(
                xt0[:], x_d[0:128, r0 * W : r1 * W].rearrange("p (r w) -> p r w", w=W)
            )
            nc.sync.dma_start(
                xt1[:], x_d[128:192, r0 * W : r1 * W].rearrange("p (r w) -> p r w", w=W)
            )

            # conv1 into padded z tiles (groups of <=4 rows)
            for g0 in range(0, nrows, 4):
                gn = min(4, nrows - g0)
                for b, (c0, c1) in enumerate(BLKS):
                    p = c1 - c0
                    ps = zps.tile([128, 512], F32, tag="zps")
                    nc.tensor.matmul(
                        ps[0:p, 0 : gn * W],
                        w1t0[:, c0:c1],
                        xt0[:, g0 : g0 + gn, :],
                        start=True,
                        stop=False,
                    )
                    nc.tensor.matmul(
                        ps[0:p, 0 : gn * W],
                        w1t1[:, c0:c1],
                        xt1[:, g0 : g0 + gn, :],
                        start=False,
                        stop=True,
                    )
                    copy(zs[b][:, brow0 + g0 : brow0 + g0 + gn, 1 : 1 + W], ps[0:p, 0 : gn * W])

            if c == NCHUNK - 1:
                # bottom halo row never written this chunk; clear stale data
                for b in range(5):
                    nc.gpsimd.memset(zs[b][:, ZROWS - 1 : ZROWS, :], 0.0)

            # taps: 9 accumulating diagonal matmuls -> qkv rows Rc..Rc+R
            st = [stp.tile([128, R // 4, 4 * W], BF16, tag=f"st{i}", name=f"st{i}") for i in range(3)]
            for g in range(R // 4):
                orow = 1 + 4 * g  # buf row of first output row in this group
                for b, (c0, c1) in enumerate(BLKS):
                    p = c1 - c0
                    ps = qps.tile([128, 512], F32, tag="qps")
                    for t, (di, dj) in enumerate(TAPS):
                        nc.tensor.matmul(
                            ps[0:p, :],
                            diagw[b][t][:],
                            zs[b][:, orow + di : orow + di + 4, 1 + dj : 1 + dj + W],
                            start=(t == 0),
                            stop=(t == 8),
                        )
                    if b < 3:
                        copy(st[b][:, g, :], ps[:, :])
                    elif b == 3:
                        copy(v0[:, c * R * W + g * 512 : c * R * W + (g + 1) * 512], ps[:, :])
                    else:
                        copy(v1[:, c * R * W + g * 512 : c * R * W + (g + 1) * 512], ps[0:64, :])

            # transpose q,k: qkt[:, lt, 0, :] = k^T, [:, lt, 1, :] = q^T
            st_flat = [s.rearrange("p a b -> p (a b)") for s in st]
            qkt = qktp.tile([128, R, 2, 192], BF16, tag="qkt")
            for lt in range(R):
                sl = slice(lt * 128, (lt + 1) * 128)
                nc.sync.dma_start_transpose(qkt[:, lt, 1, 0:128], st_flat[0][:, sl])
                nc.sync.dma_start_transpose(qkt[:, lt, 1, 128:192], st_flat[1][0:64, sl])
                nc.sync.dma_start_transpose(qkt[:, lt, 0, 0:64], st_flat[1][64:128, sl])
                nc.sync.dma_start_transpose(qkt[:, lt, 0, 64:192], st_flat[2][:, sl])

            # gram accumulation
            for lt in range(R):
                first = c == 0 and lt == 0
                last = c == NCHUNK - 1 and lt == R - 1
                for h in range(HEADS):
                    nc.tensor.matmul(
                        gh[h][:, 0:96],
                        qkt[:, lt, 1, h * DH : (h + 1) * DH],
                        qkt[:, lt, :, h * DH : (h + 1) * DH],
                        start=first,
                        stop=False,
                    )
                    nc.tensor.matmul(
                        gh[h][:, 96:144],
                        qkt[:, lt, 0, h * DH : (h + 1) * DH],
                        qkt[:, lt, 0, h * DH : (h + 1) * DH],
                        start=False,
                        stop=last,
                    )

        for h in range(HEADS):
            nc.vector.tensor_copy(ghs[:, h * 144 : (h + 1) * 144], gh[h][:])

    # ---- phase B ----
    with (
        tc.tile_pool(name="bsb", bufs=1) as bsb,
        tc.tile_pool(name="bps", bufs=1, space="PSUM") as bps,
        tc.tile_pool(name="ops", bufs=2, space="PSUM") as ops,
        tc.tile_pool(name="osb", bufs=4) as osb,
    ):
        attn_bf = bsb.tile([48, HEADS * 48], BF16, tag="attnbf")
        scr = bsb.tile([48, 48], F32, tag="scr")
        scr2 = bsb.tile([48, 48], F32, tag="scr2")
        colv = bsb.tile([48, 1], F32, tag="colv")
        rowv = bsb.tile([1, 48], F32, tag="rowv")
        rkrep = bsb.tile([48, 48], F32, tag="rkrep")
        logits = bsb.tile([48, 48], F32, tag="logits")

        for h in range(HEADS):
            gqk = ghs[:, h * 144 : h * 144 + 48]
            gqq = ghs[:, h * 144 + 48 : h * 144 + 96]
            gkk = ghs[:, h * 144 + 96 : h * 144 + 144]

            # rq_inv = 1/max(sqrt(diag(Gqq)),eps), with 1/sqrt(DH) folded in
            nc.vector.tensor_mul(scr[:], gqq, ident48[:])
            nc.vector.reduce_sum(colv[:], scr[:], axis=mybir.AxisListType.X)
            nc.scalar.activation(colv[:], colv[:], AF.Sqrt)
            nc.vector.tensor_scalar_max(colv[:], colv[:], 1e-12)
            nc.vector.reciprocal(colv[:], colv[:])
            nc.vector.tensor_scalar(
                logits[:],
                gqk,
                colv[:],
                1.0 / math.sqrt(DH),
                op0=mybir.AluOpType.mult,
                op1=mybir.AluOpType.mult,
            )

            # rk_inv broadcast along the free (key) dim via diag-as-row
            nc.vector.tensor_mul(scr2[:], gkk, ident48[:])
            ps_row = bps.tile([1, 48], F32, tag="pssmall")
            nc.tensor.matmul(ps_row[:], ones48[:], scr2[:], start=True, stop=True)
            nc.vector.tensor_copy(rowv[:], ps_row[:])
            nc.scalar.activation(rowv[:], rowv[:], AF.Sqrt)
            nc.vector.tensor_scalar_max(rowv[:], rowv[:], 1e-12)
            nc.vector.reciprocal(rowv[:], rowv[:])
            ps_rep = bps.tile([48, 48], F32, tag="pssmall")
            nc.tensor.matmul(ps_rep[:], ones1x48[:], rowv[:], start=True, stop=True)
            nc.vector.tensor_copy(rkrep[:], ps_rep[:])
            nc.vector.tensor_mul(logits[:], logits[:], rkrep[:])

            # softmax over the free (key) dim
            nc.vector.reduce_max(colv[:], logits[:], axis=mybir.AxisListType.X)
            nc.vector.tensor_scalar_sub(logits[:], logits[:], colv[:])
            nc.scalar.activation(logits[:], logits[:], AF.Exp)
            nc.vector.reduce_sum(colv[:], logits[:], axis=mybir.AxisListType.X)
            nc.vector.reciprocal(colv[:], colv[:])
            nc.vector.tensor_scalar_mul(logits[:], logits[:], colv[:])
            nc.vector.tensor_copy(attn_bf[:, h * 48 : (h + 1) * 48], logits[:])

        # block-diagonal attn (bf16)
        bd0 = bsb.tile([128, C], BF16, tag="bd0")
        bd1 = bsb.tile([64, C], BF16, tag="bd1")
        nc.gpsimd.memset(bd0[:], 0.0)
        nc.gpsimd.memset(bd1[:], 0.0)
        nc.sync.dma_start(bd0[0:48, 0:48], attn_bf[:, 0:48])
        nc.sync.dma_start(bd0[48:96, 48:96], attn_bf[:, 48:96])
        nc.sync.dma_start(bd0[96:128, 96:144], attn_bf[0:32, 96:144])
        nc.sync.dma_start(bd1[0:16, 96:144], attn_bf[32:48, 96:144])
        nc.sync.dma_start(bd1[16:64, 144:192], attn_bf[:, 144:192])

        # W_effT = BD(attn).T @ W_outT   [192 x 192], bf16
        weff0 = bsb.tile([128, C], BF16, tag="weff0")
        weff1 = bsb.tile([64, C], BF16, tag="weff1")
        for m0, m1, wt in [(0, 128, weff0), (128, 192, weff1)]:
            pw = bps.tile([128, C], F32, tag="pweff")
            nc.tensor.matmul(pw[0 : m1 - m0, :], bd0[:, m0:m1], woutt0_bf[:], start=True, stop=False)
            nc.tensor.matmul(pw[0 : m1 - m0, :], bd1[:, m0:m1], woutt1_bf[:], start=False, stop=True)
            copy(wt[:], pw[0 : m1 - m0, :])

        # y = W_effT.T @ v
        for g in range(L // 512):
            sl = slice(g * 512, (g + 1) * 512)
            for m0, m1 in [(0, 128), (128, 192)]:
                po = ops.tile([128, 512], F32, tag="ops")
                nc.tensor.matmul(po[0 : m1 - m0, :], weff0[:, m0:m1], v0[:, sl], start=True, stop=False)
                nc.tensor.matmul(po[0 : m1 - m0, :], weff1[:, m0:m1], v1[:, sl], start=False, stop=True)
                ot = osb.tile([m1 - m0, 512], F32, tag=f"o{m0}", name=f"o{m0}")
                copy(ot[:], po[0 : m1 - m0, :])
                nc.sync.dma_start(y_d[m0:m1, sl], ot[:])


_NC_CACHE = None


def _get_nc():
    global _NC_CACHE
    if _NC_CACHE is None:
        _NC_CACHE = build_nc()
    return _NC_CACHE


def kernel(x, w_proj1, w_dw, pos_emb, w_out, _trace=False):
    from concourse.bass_utils import run_bass_kernel_spmd

    x = np.asarray(x, dtype=np.float32)
    w1t = np.ascontiguousarray(np.asarray(w_proj1, np.float32).reshape(CQKV, C).T)
    wdw = np.ascontiguousarray(np.asarray(w_dw, np.float32).reshape(CQKV, 9))
    woutt = np.ascontiguousarray(np.asarray(w_out, np.float32).reshape(C, C).T)
    # pos_emb adds a per-head constant to every logit in its softmax row;
    # softmax is shift-invariant, so it has no effect on the output.

    nc = _get_nc()
    in_maps = [
        {"x": np.ascontiguousarray(x[b].reshape(C, L)), "w1t": w1t, "wdw": wdw, "woutt": woutt}
        for b in range(N_CORES)
    ]
    res = run_bass_kernel_spmd(nc, in_maps, list(range(N_CORES)), trace=_trace)
    out = np.stack([res.results[b]["y"].reshape(C, H, W) for b in range(N_CORES)])
    if _trace:
        kernel.last_exec_time_ns = res.exec_time_ns
        kernel.last_profile = res
    return out.astype(np.float32)


# revision 5
# speedup vs baseline: 1.8924x; 1.8924x over previous
"""ChannelSA Trainium2 kernel: 8-way batch-parallel across NeuronCores.

kernel(**inputs) takes the FULL inputs (x [8,192,128,128], conv weights,
pos_emb) and returns the FULL output [8,192,128,128] fp32. Each core runs
an identical single-batch program (SPMD, no collectives).

Per-core pipeline:
  z   = W1 @ x                   1x1 conv, fp32r matmuls (K=192 -> 128+64)
  qkv = DW3x3(z)                 9 accumulating diagonal matmuls on TensorE
                                 over a zero-padded bf16 z layout (shifted APs)
  q,k -> bf16 DMA-transpose ->   per-head Gram banks [Gqk|Gqq|Gkk] in PSUM
  logits = Gqk / (|q||k| sqrt(48))   norms taken from the Gram diagonals;
                                 pos_emb is constant per softmax row: a no-op
  attn = softmax(logits)
  y = (W_out @ blockdiag(attn)) @ v   output projection fused with attn@v
"""
import math
from contextlib import ExitStack

import numpy as np

import concourse.bass as bass
import concourse.mybir as mybir
import concourse.tile as tile
from concourse.masks import make_identity

F32 = mybir.dt.float32
F32R = mybir.dt.float32r
BF16 = mybir.dt.bfloat16
AF = mybir.ActivationFunctionType

C = 192
CQKV = 576
H = 128
W = 128
L = H * W
HEADS = 4
DH = 48
R = 8                    # output image rows per chunk
NCHUNK = H // R
PADW = W + 2             # padded row stride in z tiles
ZROWS = R + 2            # rows held per z chunk (1 halo each side)
TAPS = [(di, dj) for di in (-1, 0, 1) for dj in (-1, 0, 1)]
BLKS = [(0, 128), (128, 256), (256, 384), (384, 512), (512, 576)]
N_CORES = 8

_MAX_DRAIN_WAITS = 1


def _patch_tail_drain():
    """The walrus in this image rejects >1 semaphore wait on the Tile tail
    drain instruction; split the waits across a chain of SP nops."""
    if getattr(tile.TileContext, "_drain_patched", False):
        return

    def _drain_and_barrier(self, tick_clock, wait_clock):
        from concourse.vector_clock import ScopedClock

        nc = self.nc
        drain_inst = nc.sync.drain()
        wait_clock.add_sem_waits(
            drain_inst.ins, ScopedClock({None: tick_clock.global_clock})
        )
        si = drain_inst.ins.sync_info
        waits = list(si.on_wait or [])
        if len(waits) > _MAX_DRAIN_WAITS:
            si.on_wait = waits[:_MAX_DRAIN_WAITS]
            rest = waits[_MAX_DRAIN_WAITS:]
            for i in range(0, len(rest), _MAX_DRAIN_WAITS):
                nop = nc.sync.nop(nofuse=True)
                nop.ins.sync_info = mybir.SyncInfo(
                    on_wait=rest[i : i + _MAX_DRAIN_WAITS], on_update=[]
                )
        nc.all_engine_barrier()
        assert self.sems is not None
        popped = nc._tile_sem_poison_stack.pop()
        assert popped is self._sem_poison
        nc.clear_and_free_semaphores(list(self.sems.allocated().values()))
        nc.all_engine_barrier()

    tile.TileContext._drain_and_barrier = _drain_and_barrier
    tile.TileContext._drain_patched = True


def build_nc(split_waits=True):
    _patch_tail_drain()
    nc = bass.Bass("TRN2", target_bir_lowering=False, debug=False)

    x_d = nc.declare_dram_parameter("x", [C, L], F32R, isOutput=False)
    w1t_d = nc.declare_dram_parameter("w1t", [C, CQKV], F32R, isOutput=False)
    wdw_d = nc.declare_dram_parameter("wdw", [CQKV, 9], F32, isOutput=False)
    woutt_d = nc.declare_dram_parameter("woutt", [C, C], F32, isOutput=False)
    y_d = nc.declare_dram_parameter("y", [C, L], F32, isOutput=True)

    with tile.TileContext(nc) as tc, ExitStack() as ctx:
        _body(ctx, tc, x_d, w1t_d, wdw_d, woutt_d, y_d)
    if split_waits:
        # CoreSim can't run the split module (its race detector wants sem
        # updates on every inst); the split is only needed for walrus.
        _split_excess_waits(nc)
    return nc


def _split_excess_waits(nc, maxw=1):
    """This walrus build accepts only one semaphore wait per instruction.
    Move excess waits onto same-engine no-ops inserted just before the
    offending instruction (same-engine program order preserves semantics)."""
    uid = [0]
    for f in nc.m.functions:
        for bb in f.blocks:
            il = bb.instructions
            out = []
            changed = False
            for inst in il:
                si = inst.sync_info
                waits = list(si.on_wait) if si and si.on_wait else []
                if len(waits) > maxw:
                    changed = True
                    rest, keep = waits[:-maxw], waits[-maxw:]
                    for i in range(0, len(rest), maxw):
                        uid[0] += 1
                        out.append(
                            mybir.InstNoOp(
                                name=f"I-waitsplit-{uid[0]}",
                                engine=inst.engine,
                                ins=[],
                                outs=[],
                                sync_info=mybir.SyncInfo(
                                    on_wait=rest[i : i + maxw], on_update=[]
                                ),
                            )
                        )
                    si.on_wait = keep
                out.append(inst)
            if changed:
                bb.instructions = out


def _body(ctx, tc, x_d, w1t_d, wdw_d, woutt_d, y_d):
    nc = tc.nc
    ncopy = [0]

    def copy(dst, src):
        # alternate PSUM->SBUF copies between ACT and DVE
        if ncopy[0] % 2 == 0:
            nc.scalar.copy(dst, src)
        else:
            nc.vector.tensor_copy(dst, src)
        ncopy[0] += 1

    const = ctx.enter_context(tc.tile_pool(name="const", bufs=1))
    persist = ctx.enter_context(tc.tile_pool(name="persist", bufs=1))

    # ---- constants / weights ----
    w1t0 = const.tile([128, CQKV], F32R, tag="w1t0")
    w1t1 = const.tile([64, CQKV], F32R, tag="w1t1")
    nc.sync.dma_start(w1t0[:], w1t_d[0:128, :])
    nc.sync.dma_start(w1t1[:], w1t_d[128:192, :])

    woutt0 = const.tile([128, C], F32, tag="woutt0")
    woutt1 = const.tile([64, C], F32, tag="woutt1")
    nc.sync.dma_start(woutt0[:], woutt_d[0:128, :])
    nc.sync.dma_start(woutt1[:], woutt_d[128:192, :])
    woutt0_bf = const.tile([128, C], BF16, tag="woutt0bf")
    woutt1_bf = const.tile([64, C], BF16, tag="woutt1bf")
    nc.vector.tensor_copy(woutt0_bf[:], woutt0[:])
    nc.vector.tensor_copy(woutt1_bf[:], woutt1[:])

    ident_bf = const.tile([128, 128], BF16, tag="identbf")
    make_identity(nc, ident_bf[:])
    ident48 = const.tile([48, 48], F32, tag="ident48")
    make_identity(nc, ident48[:])
    ones48 = const.tile([48, 1], F32, tag="ones48")
    nc.gpsimd.memset(ones48[:], 1.0)
    ones1x48 = const.tile([1, 48], F32, tag="ones1x48")
    nc.gpsimd.memset(ones1x48[:], 1.0)

    # dw weights -> 45 diagonal bf16 matrices
    diagw = []
    for b, (c0, c1) in enumerate(BLKS):
        p = c1 - c0
        wdw_sb = const.tile([p, 9], F32, tag=f"wdw{b}")
        nc.sync.dma_start(wdw_sb[:], wdw_d[c0:c1, :])
        row = []
        for t in range(9):
            dt_ = const.tile([p, p], BF16, tag=f"diag{b}_{t}")
            nc.vector.tensor_scalar_mul(dt_[:], ident_bf[0:p, 0:p], wdw_sb[:, t : t + 1])
            row.append(dt_)
        diagw.append(row)

    # ---- persistent state ----
    v0 = persist.tile([128, L], BF16, tag="v0")
    v1 = persist.tile([64, L], BF16, tag="v1")
    zt = [
        [
            persist.tile([c1 - c0, ZROWS, PADW], BF16, tag=f"z{s}_{b}", name=f"z{s}_{b}")
            for b, (c0, c1) in enumerate(BLKS)
        ]
        for s in range(2)
    ]
    for s in range(2):
        for b in range(5):
            nc.gpsimd.memset(zt[s][b][:], 0.0)

    ghs = persist.tile([48, HEADS * 144], F32, tag="ghs")

    # ---- phase A: chunked pipeline ----
    with (
        tc.tile_pool(name="gps", bufs=1, space="PSUM") as gps,
        tc.tile_pool(name="xp", bufs=2) as xp,
        tc.tile_pool(name="zps", bufs=2, space="PSUM") as zps,
        tc.tile_pool(name="qps", bufs=2, space="PSUM") as qps,
        tc.tile_pool(name="stp", bufs=2) as stp,
        tc.tile_pool(name="qktp", bufs=2) as qktp,
    ):
        # one PSUM bank per head: [Gqk | Gqq | Gkk] (one accumulation group each)
        gh = [gps.tile([48, 144], F32, tag=f"gh{h}", name=f"gh{h}") for h in range(HEADS)]
        for c in range(NCHUNK):
            zs = zt[c % 2]
            r0 = max(0, R * c - 1)
            r1 = min(H, R * c + R + 1)
            nrows = r1 - r0
            brow0 = r0 - (R * c - 1)  # buf row of image row r0

            xt0 = xp.tile([128, nrows, W], F32R, tag="x0")
            xt1 = xp.tile([64, nrows, W], F32R, tag="x1")
            nc.sync.dma_start(
                xt0[:], x_d[0:128, r0 * W : r1 * W].rearrange("p (r w) -> p r w", w=W)
            )
            nc.sync.dma_start(
                xt1[:], x_d[128:192, r0 * W : r1 * W].rearrange("p (r w) -> p r w", w=W)
            )

            # conv1 into padded z tiles (groups of <=4 rows)
            for g0 in range(0, nrows, 4):
                gn = min(4, nrows - g0)
                for b, (c0, c1) in enumerate(BLKS):
                    p = c1 - c0
                    ps = zps.tile([128, 512], F32, tag="zps")
                    nc.tensor.matmul(
                        ps[0:p, 0 : gn * W],
                        w1t0[:, c0:c1],
                        xt0[:, g0 : g0 + gn, :],
                        start=True,
                        stop=False,
                    )
                    nc.tensor.matmul(
                        ps[0:p, 0 : gn * W],
                        w1t1[:, c0:c1],
                        xt1[:, g0 : g0 + gn, :],
                        start=False,
                        stop=True,
                    )
                    copy(zs[b][:, brow0 + g0 : brow0 + g0 + gn, 1 : 1 + W], ps[0:p, 0 : gn * W])

            if c == NCHUNK - 1:
                # bottom halo row never written this chunk; clear stale data
                for b in range(5):
                    nc.gpsimd.memset(zs[b][:, ZROWS - 1 : ZROWS, :], 0.0)

            # taps: 9 accumulating diagonal matmuls -> qkv rows Rc..Rc+R
            st = [stp.tile([128, R // 4, 4 * W], BF16, tag=f"st{i}", name=f"st{i}") for i in range(3)]
            for g in range(R // 4):
                orow = 1 + 4 * g  # buf row of first output row in this group
                for b, (c0, c1) in enumerate(BLKS):
                    p = c1 - c0
                    ps = qps.tile([128, 512], F32, tag="qps")
                    for t, (di, dj) in enumerate(TAPS):
                        nc.tensor.matmul(
                            ps[0:p, :],
                            diagw[b][t][:],
                            zs[b][:, orow + di : orow + di + 4, 1 + dj : 1 + dj + W],
                            start=(t == 0),
                            stop=(t == 8),
                        )
                    if b < 3:
                        copy(st[b][:, g, :], ps[:, :])
                    elif b == 3:
                        copy(v0[:, c * R * W + g * 512 : c * R * W + (g + 1) * 512], ps[:, :])
                    else:
                        copy(v1[:, c * R * W + g * 512 : c * R * W + (g + 1) * 512], ps[0:64, :])

            # transpose q,k: qkt[:, lt, 0, :] = k^T, [:, lt, 1, :] = q^T
            # batched 3D-out form: out[p, lt, c] = in[c, lt*128 + p]
            st_flat = [s.rearrange("p a b -> p (a b)") for s in st]
            qkt = qktp.tile([128, R, 2, 192], BF16, tag="qkt")
            nc.sync.dma_start_transpose(qkt[:, :, 1, 0:128], st_flat[0][:, :])
            nc.scalar.dma_start_transpose(qkt[:, :, 1, 128:192], st_flat[1][0:64, :])
            nc.sync.dma_start_transpose(qkt[:, :, 0, 0:64], st_flat[1][64:128, :])
            nc.scalar.dma_start_transpose(qkt[:, :, 0, 64:192], st_flat[2][:, :])

            # gram accumulation
            for lt in range(R):
                first = c == 0 and lt == 0
                last = c == NCHUNK - 1 and lt == R - 1
                for h in range(HEADS):
                    nc.tensor.matmul(
                        gh[h][:, 0:96],
                        qkt[:, lt, 1, h * DH : (h + 1) * DH],
                        qkt[:, lt, :, h * DH : (h + 1) * DH],
                        start=first,
                        stop=False,
                    )
                    nc.tensor.matmul(
                        gh[h][:, 96:144],
                        qkt[:, lt, 0, h * DH : (h + 1) * DH],
                        qkt[:, lt, 0, h * DH : (h + 1) * DH],
                        start=False,
                        stop=last,
                    )

        for h in range(HEADS):
            nc.vector.tensor_copy(ghs[:, h * 144 : (h + 1) * 144], gh[h][:])

    # ---- phase B ----
    with (
        tc.tile_pool(name="bsb", bufs=1) as bsb,
        tc.tile_pool(name="bps", bufs=1, space="PSUM") as bps,
        tc.tile_pool(name="ops", bufs=2, space="PSUM") as ops,
        tc.tile_pool(name="osb", bufs=4) as osb,
    ):
        attn_bf = bsb.tile([48, HEADS * 48], BF16, tag="attnbf")
        scr = bsb.tile([48, 48], F32, tag="scr")
        scr2 = bsb.tile([48, 48], F32, tag="scr2")
        colv = bsb.tile([48, 1], F32, tag="colv")
        rowv = bsb.tile([1, 48], F32, tag="rowv")
        rkrep = bsb.tile([48, 48], F32, tag="rkrep")
        logits = bsb.tile([48, 48], F32, tag="logits")

        for h in range(HEADS):
            gqk = ghs[:, h * 144 : h * 144 + 48]
            gqq = ghs[:, h * 144 + 48 : h * 144 + 96]
            gkk = ghs[:, h * 144 + 96 : h * 144 + 144]

            # rq_inv = 1/max(sqrt(diag(Gqq)),eps), with 1/sqrt(DH) folded in
            nc.vector.tensor_mul(scr[:], gqq, ident48[:])
            nc.vector.reduce_sum(colv[:], scr[:], axis=mybir.AxisListType.X)
            nc.scalar.activation(colv[:], colv[:], AF.Sqrt)
            nc.vector.tensor_scalar_max(colv[:], colv[:], 1e-12)
            nc.vector.reciprocal(colv[:], colv[:])
            nc.vector.tensor_scalar(
                logits[:],
                gqk,
                colv[:],
                1.0 / math.sqrt(DH),
                op0=mybir.AluOpType.mult,
                op1=mybir.AluOpType.mult,
            )

            # rk_inv broadcast along the free (key) dim via diag-as-row
            nc.vector.tensor_mul(scr2[:], gkk, ident48[:])
            ps_row = bps.tile([1, 48], F32, tag="pssmall")
            nc.tensor.matmul(ps_row[:], ones48[:], scr2[:], start=True, stop=True)
            nc.vector.tensor_copy(rowv[:], ps_row[:])
            nc.scalar.activation(rowv[:], rowv[:], AF.Sqrt)
            nc.vector.tensor_scalar_max(rowv[:], rowv[:], 1e-12)
            nc.vector.reciprocal(rowv[:], rowv[:])
            ps_rep = bps.tile([48, 48], F32, tag="pssmall")
            nc.tensor.matmul(ps_rep[:], ones1x48[:], rowv[:], start=True, stop=True)
            nc.vector.tensor_copy(rkrep[:], ps_rep[:])
            nc.vector.tensor_mul(logits[:], logits[:], rkrep[:])

            # softmax over the free (key) dim
            nc.vector.reduce_max(colv[:], logits[:], axis=mybir.AxisListType.X)
            nc.vector.tensor_scalar_sub(logits[:], logits[:], colv[:])
            nc.scalar.activation(logits[:], logits[:], AF.Exp)
            nc.vector.reduce_sum(colv[:], logits[:], axis=mybir.AxisListType.X)
            nc.vector.reciprocal(colv[:], colv[:])
            nc.vector.tensor_scalar_mul(logits[:], logits[:], colv[:])
            nc.vector.tensor_copy(attn_bf[:, h * 48 : (h + 1) * 48], logits[:])

        # block-diagonal attn (bf16)
        bd0 = bsb.tile([128, C], BF16, tag="bd0")
        bd1 = bsb.tile([64, C], BF16, tag="bd1")
        nc.gpsimd.memset(bd0[:], 0.0)
        nc.gpsimd.memset(bd1[:], 0.0)
        nc.sync.dma_start(bd0[0:48, 0:48], attn_bf[:, 0:48])
        nc.sync.dma_start(bd0[48:96, 48:96], attn_bf[:, 48:96])
        nc.sync.dma_start(bd0[96:128, 96:144], attn_bf[0:32, 96:144])
        nc.sync.dma_start(bd1[0:16, 96:144], attn_bf[32:48, 96:144])
        nc.sync.dma_start(bd1[16:64, 144:192], attn_bf[:, 144:192])

        # W_effT = BD(attn).T @ W_outT   [192 x 192], bf16
        weff0 = bsb.tile([128, C], BF16, tag="weff0")
        weff1 = bsb.tile([64, C], BF16, tag="weff1")
        for m0, m1, wt in [(0, 128, weff0), (128, 192, weff1)]:
            pw = bps.tile([128, C], F32, tag="pweff")
            nc.tensor.matmul(pw[0 : m1 - m0, :], bd0[:, m0:m1], woutt0_bf[:], start=True, stop=False)
            nc.tensor.matmul(pw[0 : m1 - m0, :], bd1[:, m0:m1], woutt1_bf[:], start=False, stop=True)
            copy(wt[:], pw[0 : m1 - m0, :])

        # y = W_effT.T @ v
        for g in range(L // 512):
            sl = slice(g * 512, (g + 1) * 512)
            for m0, m1 in [(0, 128), (128, 192)]:
                po = ops.tile([128, 512], F32, tag="ops")
                nc.tensor.matmul(po[0 : m1 - m0, :], weff0[:, m0:m1], v0[:, sl], start=True, stop=False)
                nc.tensor.matmul(po[0 : m1 - m0, :], weff1[:, m0:m1], v1[:, sl], start=False, stop=True)
                ot = osb.tile([m1 - m0, 512], F32, tag=f"o{m0}", name=f"o{m0}")
                copy(ot[:], po[0 : m1 - m0, :])
                nc.sync.dma_start(y_d[m0:m1, sl], ot[:])


_NC_CACHE = None


def _get_nc():
    global _NC_CACHE
    if _NC_CACHE is None:
        _NC_CACHE = build_nc()
    return _NC_CACHE


def kernel(x, w_proj1, w_dw, pos_emb, w_out, _trace=False):
    from concourse.bass_utils import run_bass_kernel_spmd

    x = np.asarray(x, dtype=np.float32)
    w1t = np.ascontiguousarray(np.asarray(w_proj1, np.float32).reshape(CQKV, C).T)
    wdw = np.ascontiguousarray(np.asarray(w_dw, np.float32).reshape(CQKV, 9))
    woutt = np.ascontiguousarray(np.asarray(w_out, np.float32).reshape(C, C).T)
    # pos_emb adds a per-head constant to every logit in its softmax row;
    # softmax is shift-invariant, so it has no effect on the output.

    nc = _get_nc()
    in_maps = [
        {"x": np.ascontiguousarray(x[b].reshape(C, L)), "w1t": w1t, "wdw": wdw, "woutt": woutt}
        for b in range(N_CORES)
    ]
    res = run_bass_kernel_spmd(nc, in_maps, list(range(N_CORES)), trace=_trace)
    out = np.stack([res.results[b]["y"].reshape(C, H, W) for b in range(N_CORES)])
    if _trace:
        kernel.last_exec_time_ns = res.exec_time_ns
        kernel.last_profile = res
    return out.astype(np.float32)


# revision 6
# speedup vs baseline: 2.0124x; 1.0634x over previous
"""ChannelSA Trainium2 kernel: 8-way batch-parallel across NeuronCores.

kernel(**inputs) takes the FULL inputs (x [8,192,128,128], conv weights,
pos_emb) and returns the FULL output [8,192,128,128] fp32. Each core runs
an identical single-batch program (SPMD, no collectives).

Per-core pipeline:
  z   = W1 @ x                   1x1 conv, fp32r matmuls (K=192 -> 128+64)
  qkv = DW3x3(z)                 9 accumulating diagonal matmuls on TensorE
                                 over a zero-padded bf16 z layout (shifted APs)
  q,k -> bf16 DMA-transpose ->   per-head Gram banks [Gqk|Gqq|Gkk] in PSUM
  logits = Gqk / (|q||k| sqrt(48))   norms taken from the Gram diagonals;
                                 pos_emb is constant per softmax row: a no-op
  attn = softmax(logits)
  y = (W_out @ blockdiag(attn)) @ v   output projection fused with attn@v
"""
import math
from contextlib import ExitStack

import numpy as np

import concourse.bass as bass
import concourse.mybir as mybir
import concourse.tile as tile
from concourse.masks import make_identity

F32 = mybir.dt.float32
F32R = mybir.dt.float32r
BF16 = mybir.dt.bfloat16
AF = mybir.ActivationFunctionType

C = 192
CQKV = 576
H = 128
W = 128
L = H * W
HEADS = 4
DH = 48
R = 8                    # output image rows per chunk
NCHUNK = H // R
PADW = W + 2             # padded row stride in z tiles
ZROWS = R + 2            # rows held per z chunk (1 halo each side)
TAPS = [(di, dj) for di in (-1, 0, 1) for dj in (-1, 0, 1)]
BLKS = [(0, 128), (128, 256), (256, 384), (384, 512), (512, 576)]
N_CORES = 8

_MAX_DRAIN_WAITS = 1


def _patch_tail_drain():
    """The walrus in this image rejects >1 semaphore wait on the Tile tail
    drain instruction; split the waits across a chain of SP nops."""
    if getattr(tile.TileContext, "_drain_patched", False):
        return

    def _drain_and_barrier(self, tick_clock, wait_clock):
        from concourse.vector_clock import ScopedClock

        nc = self.nc
        drain_inst = nc.sync.drain()
        wait_clock.add_sem_waits(
            drain_inst.ins, ScopedClock({None: tick_clock.global_clock})
        )
        si = drain_inst.ins.sync_info
        waits = list(si.on_wait or [])
        if len(waits) > _MAX_DRAIN_WAITS:
            si.on_wait = waits[:_MAX_DRAIN_WAITS]
            rest = waits[_MAX_DRAIN_WAITS:]
            for i in range(0, len(rest), _MAX_DRAIN_WAITS):
                nop = nc.sync.nop(nofuse=True)
                nop.ins.sync_info = mybir.SyncInfo(
                    on_wait=rest[i : i + _MAX_DRAIN_WAITS], on_update=[]
                )
        nc.all_engine_barrier()
        assert self.sems is not None
        popped = nc._tile_sem_poison_stack.pop()
        assert popped is self._sem_poison
        nc.clear_and_free_semaphores(list(self.sems.allocated().values()))
        nc.all_engine_barrier()

    tile.TileContext._drain_and_barrier = _drain_and_barrier
    tile.TileContext._drain_patched = True


def build_nc(split_waits=True):
    _patch_tail_drain()
    nc = bass.Bass("TRN2", target_bir_lowering=False, debug=False)

    x_d = nc.declare_dram_parameter("x", [C, L], BF16, isOutput=False)
    w1t_d = nc.declare_dram_parameter("w1t", [C, CQKV], BF16, isOutput=False)
    wdw_d = nc.declare_dram_parameter("wdw", [CQKV, 9], F32, isOutput=False)
    woutt_d = nc.declare_dram_parameter("woutt", [C, C], F32, isOutput=False)
    y_d = nc.declare_dram_parameter("y", [C, L], F32, isOutput=True)

    with tile.TileContext(nc) as tc, ExitStack() as ctx:
        _body(ctx, tc, x_d, w1t_d, wdw_d, woutt_d, y_d)
    if split_waits:
        # CoreSim can't run the split module (its race detector wants sem
        # updates on every inst); the split is only needed for walrus.
        _split_excess_waits(nc)
    return nc


def _split_excess_waits(nc, maxw=1):
    """This walrus build accepts only one semaphore wait per instruction.
    Move excess waits onto same-engine no-ops inserted just before the
    offending instruction (same-engine program order preserves semantics)."""
    uid = [0]
    for f in nc.m.functions:
        for bb in f.blocks:
            il = bb.instructions
            out = []
            changed = False
            for inst in il:
                si = inst.sync_info
                waits = list(si.on_wait) if si and si.on_wait else []
                if len(waits) > maxw:
                    changed = True
                    rest, keep = waits[:-maxw], waits[-maxw:]
                    for i in range(0, len(rest), maxw):
                        uid[0] += 1
                        out.append(
                            mybir.InstNoOp(
                                name=f"I-waitsplit-{uid[0]}",
                                engine=inst.engine,
                                ins=[],
                                outs=[],
                                sync_info=mybir.SyncInfo(
                                    on_wait=rest[i : i + maxw], on_update=[]
                                ),
                            )
                        )
                    si.on_wait = keep
                out.append(inst)
            if changed:
                bb.instructions = out


def _body(ctx, tc, x_d, w1t_d, wdw_d, woutt_d, y_d):
    nc = tc.nc
    ncopy = [0]

    def copy(dst, src):
        # alternate PSUM->SBUF copies between ACT and DVE
        if ncopy[0] % 2 == 0:
            nc.scalar.copy(dst, src)
        else:
            nc.vector.tensor_copy(dst, src)
        ncopy[0] += 1

    const = ctx.enter_context(tc.tile_pool(name="const", bufs=1))
    persist = ctx.enter_context(tc.tile_pool(name="persist", bufs=1))

    # ---- constants / weights ----
    w1t0 = const.tile([128, CQKV], BF16, tag="w1t0")
    w1t1 = const.tile([64, CQKV], BF16, tag="w1t1")
    nc.sync.dma_start(w1t0[:], w1t_d[0:128, :])
    nc.sync.dma_start(w1t1[:], w1t_d[128:192, :])

    woutt0 = const.tile([128, C], F32, tag="woutt0")
    woutt1 = const.tile([64, C], F32, tag="woutt1")
    nc.sync.dma_start(woutt0[:], woutt_d[0:128, :])
    nc.sync.dma_start(woutt1[:], woutt_d[128:192, :])
    woutt0_bf = const.tile([128, C], BF16, tag="woutt0bf")
    woutt1_bf = const.tile([64, C], BF16, tag="woutt1bf")
    nc.vector.tensor_copy(woutt0_bf[:], woutt0[:])
    nc.vector.tensor_copy(woutt1_bf[:], woutt1[:])

    ident_bf = const.tile([128, 128], BF16, tag="identbf")
    make_identity(nc, ident_bf[:])
    ident48 = const.tile([48, 48], F32, tag="ident48")
    make_identity(nc, ident48[:])
    ones48 = const.tile([48, 1], F32, tag="ones48")
    nc.gpsimd.memset(ones48[:], 1.0)
    ones1x48 = const.tile([1, 48], F32, tag="ones1x48")
    nc.gpsimd.memset(ones1x48[:], 1.0)

    # dw weights -> 45 diagonal bf16 matrices
    diagw = []
    for b, (c0, c1) in enumerate(BLKS):
        p = c1 - c0
        wdw_sb = const.tile([p, 9], F32, tag=f"wdw{b}")
        nc.sync.dma_start(wdw_sb[:], wdw_d[c0:c1, :])
        row = []
        for t in range(9):
            dt_ = const.tile([p, p], BF16, tag=f"diag{b}_{t}")
            nc.vector.tensor_scalar_mul(dt_[:], ident_bf[0:p, 0:p], wdw_sb[:, t : t + 1])
            row.append(dt_)
        diagw.append(row)

    # ---- persistent state ----
    v0 = persist.tile([128, L], BF16, tag="v0")
    v1 = persist.tile([64, L], BF16, tag="v1")
    zt = [
        [
            persist.tile([c1 - c0, ZROWS, PADW], BF16, tag=f"z{s}_{b}", name=f"z{s}_{b}")
            for b, (c0, c1) in enumerate(BLKS)
        ]
        for s in range(2)
    ]
    for s in range(2):
        for b in range(5):
            nc.gpsimd.memset(zt[s][b][:], 0.0)

    ghs = persist.tile([48, HEADS * 144], F32, tag="ghs")

    # ---- phase A: chunked pipeline ----
    with (
        tc.tile_pool(name="gps", bufs=1, space="PSUM") as gps,
        tc.tile_pool(name="xp", bufs=2) as xp,
        tc.tile_pool(name="zps", bufs=2, space="PSUM") as zps,
        tc.tile_pool(name="qps", bufs=2, space="PSUM") as qps,
        tc.tile_pool(name="stp", bufs=2) as stp,
        tc.tile_pool(name="qktp", bufs=2) as qktp,
    ):
        # one PSUM bank per head: [Gqk | Gqq | Gkk] (one accumulation group each)
        gh = [gps.tile([48, 144], F32, tag=f"gh{h}", name=f"gh{h}") for h in range(HEADS)]
        for c in range(NCHUNK):
            zs = zt[c % 2]
            r0 = max(0, R * c - 1)
            r1 = min(H, R * c + R + 1)
            nrows = r1 - r0
            brow0 = r0 - (R * c - 1)  # buf row of image row r0

            xt0 = xp.tile([128, nrows, W], BF16, tag="x0")
            xt1 = xp.tile([64, nrows, W], BF16, tag="x1")
            nc.sync.dma_start(
                xt0[:], x_d[0:128, r0 * W : r1 * W].rearrange("p (r w) -> p r w", w=W)
            )
            nc.sync.dma_start(
                xt1[:], x_d[128:192, r0 * W : r1 * W].rearrange("p (r w) -> p r w", w=W)
            )

            # conv1 into padded z tiles (groups of <=4 rows)
            for g0 in range(0, nrows, 4):
                gn = min(4, nrows - g0)
                for b, (c0, c1) in enumerate(BLKS):
                    p = c1 - c0
                    ps = zps.tile([128, 512], F32, tag="zps")
                    nc.tensor.matmul(
                        ps[0:p, 0 : gn * W],
                        w1t0[:, c0:c1],
                        xt0[:, g0 : g0 + gn, :],
                        start=True,
                        stop=False,
                    )
                    nc.tensor.matmul(
                        ps[0:p, 0 : gn * W],
                        w1t1[:, c0:c1],
                        xt1[:, g0 : g0 + gn, :],
                        start=False,
                        stop=True,
                    )
                    copy(zs[b][:, brow0 + g0 : brow0 + g0 + gn, 1 : 1 + W], ps[0:p, 0 : gn * W])

            if c == NCHUNK - 1:
                # bottom halo row never written this chunk; clear stale data
                for b in range(5):
                    nc.gpsimd.memset(zs[b][:, ZROWS - 1 : ZROWS, :], 0.0)

            # taps: 9 accumulating diagonal matmuls -> qkv rows Rc..Rc+R
            st = [stp.tile([128, R // 4, 4 * W], BF16, tag=f"st{i}", name=f"st{i}") for i in range(3)]
            for g in range(R // 4):
                orow = 1 + 4 * g  # buf row of first output row in this group
                for b, (c0, c1) in enumerate(BLKS):
                    p = c1 - c0
                    ps = qps.tile([128, 512], F32, tag="qps")
                    for t, (di, dj) in enumerate(TAPS):
                        nc.tensor.matmul(
                            ps[0:p, :],
                            diagw[b][t][:],
                            zs[b][:, orow + di : orow + di + 4, 1 + dj : 1 + dj + W],
                            start=(t == 0),
                            stop=(t == 8),
                        )
                    if b < 3:
                        copy(st[b][:, g, :], ps[:, :])
                    elif b == 3:
                        copy(v0[:, c * R * W + g * 512 : c * R * W + (g + 1) * 512], ps[:, :])
                    else:
                        copy(v1[:, c * R * W + g * 512 : c * R * W + (g + 1) * 512], ps[0:64, :])

            # transpose q,k: qkt[:, lt, 0, :] = k^T, [:, lt, 1, :] = q^T
            # batched 3D-out form: out[p, lt, c] = in[c, lt*128 + p]
            st_flat = [s.rearrange("p a b -> p (a b)") for s in st]
            qkt = qktp.tile([128, R, 2, 192], BF16, tag="qkt")
            nc.sync.dma_start_transpose(qkt[:, :, 1, 0:128], st_flat[0][:, :])
            nc.scalar.dma_start_transpose(qkt[:, :, 1, 128:192], st_flat[1][0:64, :])
            nc.sync.dma_start_transpose(qkt[:, :, 0, 0:64], st_flat[1][64:128, :])
            nc.scalar.dma_start_transpose(qkt[:, :, 0, 64:192], st_flat[2][:, :])

            # gram accumulation
            for lt in range(R):
                first = c == 0 and lt == 0
                last = c == NCHUNK - 1 and lt == R - 1
                for h in range(HEADS):
                    nc.tensor.matmul(
                        gh[h][:, 0:96],
                        qkt[:, lt, 1, h * DH : (h + 1) * DH],
                        qkt[:, lt, :, h * DH : (h + 1) * DH],
                        start=first,
                        stop=False,
                    )
                    nc.tensor.matmul(
                        gh[h][:, 96:144],
                        qkt[:, lt, 0, h * DH : (h + 1) * DH],
                        qkt[:, lt, 0, h * DH : (h + 1) * DH],
                        start=False,
                        stop=last,
                    )

        for h in range(HEADS):
            nc.vector.tensor_copy(ghs[:, h * 144 : (h + 1) * 144], gh[h][:])

    # ---- phase B ----
    with (
        tc.tile_pool(name="bsb", bufs=1) as bsb,
        tc.tile_pool(name="bps", bufs=1, space="PSUM") as bps,
        tc.tile_pool(name="ops", bufs=2, space="PSUM") as ops,
        tc.tile_pool(name="osb", bufs=4) as osb,
    ):
        attn_bf = bsb.tile([48, HEADS * 48], BF16, tag="attnbf")
        scr = bsb.tile([48, 48], F32, tag="scr")
        scr2 = bsb.tile([48, 48], F32, tag="scr2")
        colv = bsb.tile([48, 1], F32, tag="colv")
        rowv = bsb.tile([1, 48], F32, tag="rowv")
        rkrep = bsb.tile([48, 48], F32, tag="rkrep")
        logits = bsb.tile([48, 48], F32, tag="logits")

        for h in range(HEADS):
            gqk = ghs[:, h * 144 : h * 144 + 48]
            gqq = ghs[:, h * 144 + 48 : h * 144 + 96]
            gkk = ghs[:, h * 144 + 96 : h * 144 + 144]

            # rq_inv = 1/max(sqrt(diag(Gqq)),eps), with 1/sqrt(DH) folded in
            nc.vector.tensor_mul(scr[:], gqq, ident48[:])
            nc.vector.reduce_sum(colv[:], scr[:], axis=mybir.AxisListType.X)
            nc.scalar.activation(colv[:], colv[:], AF.Sqrt)
            nc.vector.tensor_scalar_max(colv[:], colv[:], 1e-12)
            nc.vector.reciprocal(colv[:], colv[:])
            nc.vector.tensor_scalar(
                logits[:],
                gqk,
                colv[:],
                1.0 / math.sqrt(DH),
                op0=mybir.AluOpType.mult,
                op1=mybir.AluOpType.mult,
            )

            # rk_inv broadcast along the free (key) dim via diag-as-row
            nc.vector.tensor_mul(scr2[:], gkk, ident48[:])
            ps_row = bps.tile([1, 48], F32, tag="pssmall")
            nc.tensor.matmul(ps_row[:], ones48[:], scr2[:], start=True, stop=True)
            nc.vector.tensor_copy(rowv[:], ps_row[:])
            nc.scalar.activation(rowv[:], rowv[:], AF.Sqrt)
            nc.vector.tensor_scalar_max(rowv[:], rowv[:], 1e-12)
            nc.vector.reciprocal(rowv[:], rowv[:])
            ps_rep = bps.tile([48, 48], F32, tag="pssmall")
            nc.tensor.matmul(ps_rep[:], ones1x48[:], rowv[:], start=True, stop=True)
            nc.vector.tensor_copy(rkrep[:], ps_rep[:])
            nc.vector.tensor_mul(logits[:], logits[:], rkrep[:])

            # softmax over the free (key) dim
            nc.vector.reduce_max(colv[:], logits[:], axis=mybir.AxisListType.X)
            nc.vector.tensor_scalar_sub(logits[:], logits[:], colv[:])
            nc.scalar.activation(logits[:], logits[:], AF.Exp)
            nc.vector.reduce_sum(colv[:], logits[:], axis=mybir.AxisListType.X)
            nc.vector.reciprocal(colv[:], colv[:])
            nc.vector.tensor_scalar_mul(logits[:], logits[:], colv[:])
            nc.vector.tensor_copy(attn_bf[:, h * 48 : (h + 1) * 48], logits[:])

        # block-diagonal attn (bf16)
        bd0 = bsb.tile([128, C], BF16, tag="bd0")
        bd1 = bsb.tile([64, C], BF16, tag="bd1")
        nc.gpsimd.memset(bd0[:], 0.0)
        nc.gpsimd.memset(bd1[:], 0.0)
        nc.sync.dma_start(bd0[0:48, 0:48], attn_bf[:, 0:48])
        nc.sync.dma_start(bd0[48:96, 48:96], attn_bf[:, 48:96])
        nc.sync.dma_start(bd0[96:128, 96:144], attn_bf[0:32, 96:144])
        nc.sync.dma_start(bd1[0:16, 96:144], attn_bf[32:48, 96:144])
        nc.sync.dma_start(bd1[16:64, 144:192], attn_bf[:, 144:192])

        # W_effT = BD(attn).T @ W_outT   [192 x 192], bf16
        weff0 = bsb.tile([128, C], BF16, tag="weff0")
        weff1 = bsb.tile([64, C], BF16, tag="weff1")
        for m0, m1, wt in [(0, 128, weff0), (128, 192, weff1)]:
            pw = bps.tile([128, C], F32, tag="pweff")
            nc.tensor.matmul(pw[0 : m1 - m0, :], bd0[:, m0:m1], woutt0_bf[:], start=True, stop=False)
            nc.tensor.matmul(pw[0 : m1 - m0, :], bd1[:, m0:m1], woutt1_bf[:], start=False, stop=True)
            copy(wt[:], pw[0 : m1 - m0, :])

        # y = W_effT.T @ v
        for g in range(L // 512):
            sl = slice(g * 512, (g + 1) * 512)
            for m0, m1 in [(0, 128), (128, 192)]:
                po = ops.tile([128, 512], F32, tag="ops")
                nc.tensor.matmul(po[0 : m1 - m0, :], weff0[:, m0:m1], v0[:, sl], start=True, stop=False)
                nc.tensor.matmul(po[0 : m1 - m0, :], weff1[:, m0:m1], v1[:, sl], start=False, stop=True)
                ot = osb.tile([m1 - m0, 512], F32, tag=f"o{m0}", name=f"o{m0}")
                copy(ot[:], po[0 : m1 - m0, :])
                nc.sync.dma_start(y_d[m0:m1, sl], ot[:])


_NC_CACHE = None


def _get_nc():
    global _NC_CACHE
    if _NC_CACHE is None:
        _NC_CACHE = build_nc()
    return _NC_CACHE


def kernel(x, w_proj1, w_dw, pos_emb, w_out, _trace=False):
    from concourse.bass_utils import run_bass_kernel_spmd

    import ml_dtypes

    x = np.asarray(x, dtype=np.float32).astype(ml_dtypes.bfloat16)
    w1t = np.ascontiguousarray(
        np.asarray(w_proj1, np.float32).reshape(CQKV, C).T.astype(ml_dtypes.bfloat16)
    )
    wdw = np.ascontiguousarray(np.asarray(w_dw, np.float32).reshape(CQKV, 9))
    woutt = np.ascontiguousarray(np.asarray(w_out, np.float32).reshape(C, C).T)
    # pos_emb adds a per-head constant to every logit in its softmax row;
    # softmax is shift-invariant, so it has no effect on the output.

    nc = _get_nc()
    in_maps = [
        {"x": np.ascontiguousarray(x[b].reshape(C, L)), "w1t": w1t, "wdw": wdw, "woutt": woutt}
        for b in range(N_CORES)
    ]
    res = run_bass_kernel_spmd(nc, in_maps, list(range(N_CORES)), trace=_trace)
    out = np.stack([res.results[b]["y"].reshape(C, H, W) for b in range(N_CORES)])
    if _trace:
        kernel.last_exec_time_ns = res.exec_time_ns
        kernel.last_profile = res
    return out.astype(np.float32)


# revision 7
# speedup vs baseline: 2.0344x; 1.0109x over previous
"""ChannelSA Trainium2 kernel: 8-way batch-parallel across NeuronCores.

kernel(**inputs) takes the FULL inputs (x [8,192,128,128], conv weights,
pos_emb) and returns the FULL output [8,192,128,128] fp32. Each core runs
an identical single-batch program (SPMD, no collectives).

Per-core pipeline:
  z   = W1 @ x                   1x1 conv, fp32r matmuls (K=192 -> 128+64)
  qkv = DW3x3(z)                 9 accumulating diagonal matmuls on TensorE
                                 over a zero-padded bf16 z layout (shifted APs)
  q,k -> bf16 DMA-transpose ->   per-head Gram banks [Gqk|Gqq|Gkk] in PSUM
  logits = Gqk / (|q||k| sqrt(48))   norms taken from the Gram diagonals;
                                 pos_emb is constant per softmax row: a no-op
  attn = softmax(logits)
  y = (W_out @ blockdiag(attn)) @ v   output projection fused with attn@v
"""
import math
from contextlib import ExitStack

import numpy as np

import concourse.bass as bass
import concourse.mybir as mybir
import concourse.tile as tile
from concourse.masks import make_identity

F32 = mybir.dt.float32
F32R = mybir.dt.float32r
BF16 = mybir.dt.bfloat16
AF = mybir.ActivationFunctionType

C = 192
CQKV = 576
H = 128
W = 128
L = H * W
HEADS = 4
DH = 48
R = 8                    # output image rows per chunk
NCHUNK = H // R
PADW = W + 2             # padded row stride in z tiles
ZROWS = R + 2            # rows held per z chunk (1 halo each side)
TAPS = [(di, dj) for di in (-1, 0, 1) for dj in (-1, 0, 1)]
BLKS = [(0, 128), (128, 256), (256, 384), (384, 512), (512, 576)]
N_CORES = 8

_MAX_DRAIN_WAITS = 1


def _patch_tail_drain():
    """The walrus in this image rejects >1 semaphore wait on the Tile tail
    drain instruction; split the waits across a chain of SP nops."""
    if getattr(tile.TileContext, "_drain_patched", False):
        return

    def _drain_and_barrier(self, tick_clock, wait_clock):
        from concourse.vector_clock import ScopedClock

        nc = self.nc
        drain_inst = nc.sync.drain()
        wait_clock.add_sem_waits(
            drain_inst.ins, ScopedClock({None: tick_clock.global_clock})
        )
        si = drain_inst.ins.sync_info
        waits = list(si.on_wait or [])
        if len(waits) > _MAX_DRAIN_WAITS:
            si.on_wait = waits[:_MAX_DRAIN_WAITS]
            rest = waits[_MAX_DRAIN_WAITS:]
            for i in range(0, len(rest), _MAX_DRAIN_WAITS):
                nop = nc.sync.nop(nofuse=True)
                nop.ins.sync_info = mybir.SyncInfo(
                    on_wait=rest[i : i + _MAX_DRAIN_WAITS], on_update=[]
                )
        nc.all_engine_barrier()
        assert self.sems is not None
        popped = nc._tile_sem_poison_stack.pop()
        assert popped is self._sem_poison
        nc.clear_and_free_semaphores(list(self.sems.allocated().values()))
        nc.all_engine_barrier()

    tile.TileContext._drain_and_barrier = _drain_and_barrier
    tile.TileContext._drain_patched = True


def build_nc(split_waits=True):
    _patch_tail_drain()
    nc = bass.Bass("TRN2", target_bir_lowering=False, debug=False)

    x_d = nc.declare_dram_parameter("x", [C, L], BF16, isOutput=False)
    w1t_d = nc.declare_dram_parameter("w1t", [C, CQKV], BF16, isOutput=False)
    wdw_d = nc.declare_dram_parameter("wdw", [CQKV, 9], F32, isOutput=False)
    woutt_d = nc.declare_dram_parameter("woutt", [C, C], F32, isOutput=False)
    y_d = nc.declare_dram_parameter("y", [C, L], F32, isOutput=True)

    with tile.TileContext(nc) as tc, ExitStack() as ctx:
        _body(ctx, tc, x_d, w1t_d, wdw_d, woutt_d, y_d)
    if split_waits:
        # CoreSim can't run the split module (its race detector wants sem
        # updates on every inst); the split is only needed for walrus.
        _split_excess_waits(nc)
    return nc


def _split_excess_waits(nc, maxw=1):
    """This walrus build accepts only one semaphore wait per instruction.
    Move excess waits onto same-engine no-ops inserted just before the
    offending instruction (same-engine program order preserves semantics)."""
    uid = [0]
    for f in nc.m.functions:
        for bb in f.blocks:
            il = bb.instructions
            out = []
            changed = False
            for inst in il:
                si = inst.sync_info
                waits = list(si.on_wait) if si and si.on_wait else []
                if len(waits) > maxw:
                    changed = True
                    rest, keep = waits[:-maxw], waits[-maxw:]
                    for i in range(0, len(rest), maxw):
                        uid[0] += 1
                        out.append(
                            mybir.InstNoOp(
                                name=f"I-waitsplit-{uid[0]}",
                                engine=inst.engine,
                                ins=[],
                                outs=[],
                                sync_info=mybir.SyncInfo(
                                    on_wait=rest[i : i + maxw], on_update=[]
                                ),
                            )
                        )
                    si.on_wait = keep
                out.append(inst)
            if changed:
                bb.instructions = out


def _body(ctx, tc, x_d, w1t_d, wdw_d, woutt_d, y_d):
    nc = tc.nc
    ncopy = [0]

    def copy(dst, src):
        # alternate PSUM->SBUF copies between ACT and DVE
        if ncopy[0] % 2 == 0:
            nc.scalar.copy(dst, src)
        else:
            nc.vector.tensor_copy(dst, src)
        ncopy[0] += 1

    const = ctx.enter_context(tc.tile_pool(name="const", bufs=1))
    persist = ctx.enter_context(tc.tile_pool(name="persist", bufs=1))

    # ---- constants / weights ----
    w1t0 = const.tile([128, CQKV], BF16, tag="w1t0")
    w1t1 = const.tile([64, CQKV], BF16, tag="w1t1")
    nc.sync.dma_start(w1t0[:], w1t_d[0:128, :])
    nc.sync.dma_start(w1t1[:], w1t_d[128:192, :])

    woutt0 = const.tile([128, C], F32, tag="woutt0")
    woutt1 = const.tile([64, C], F32, tag="woutt1")
    nc.sync.dma_start(woutt0[:], woutt_d[0:128, :])
    nc.sync.dma_start(woutt1[:], woutt_d[128:192, :])
    woutt0_bf = const.tile([128, C], BF16, tag="woutt0bf")
    woutt1_bf = const.tile([64, C], BF16, tag="woutt1bf")
    nc.vector.tensor_copy(woutt0_bf[:], woutt0[:])
    nc.vector.tensor_copy(woutt1_bf[:], woutt1[:])

    ident_bf = const.tile([128, 128], BF16, tag="identbf")
    make_identity(nc, ident_bf[:])
    ident48 = const.tile([48, 48], F32, tag="ident48")
    make_identity(nc, ident48[:])
    ones48 = const.tile([48, 1], F32, tag="ones48")
    nc.gpsimd.memset(ones48[:], 1.0)
    ones1x48 = const.tile([1, 48], F32, tag="ones1x48")
    nc.gpsimd.memset(ones1x48[:], 1.0)

    # dw weights -> 45 diagonal bf16 matrices
    diagw = []
    for b, (c0, c1) in enumerate(BLKS):
        p = c1 - c0
        wdw_sb = const.tile([p, 9], F32, tag=f"wdw{b}")
        nc.sync.dma_start(wdw_sb[:], wdw_d[c0:c1, :])
        row = []
        for t in range(9):
            dt_ = const.tile([p, p], BF16, tag=f"diag{b}_{t}")
            nc.vector.tensor_scalar_mul(dt_[:], ident_bf[0:p, 0:p], wdw_sb[:, t : t + 1])
            row.append(dt_)
        diagw.append(row)

    # ---- persistent state ----
    v0 = persist.tile([128, L], BF16, tag="v0")
    v1 = persist.tile([64, L], BF16, tag="v1")
    zt = [
        [
            persist.tile([c1 - c0, ZROWS, PADW], BF16, tag=f"z{s}_{b}", name=f"z{s}_{b}")
            for b, (c0, c1) in enumerate(BLKS)
        ]
        for s in range(2)
    ]
    for s in range(2):
        for b in range(5):
            nc.gpsimd.memset(zt[s][b][:], 0.0)

    ghs = persist.tile([48, HEADS * 144], F32, tag="ghs")

    # ---- phase A: chunked pipeline ----
    with (
        tc.tile_pool(name="gps", bufs=1, space="PSUM") as gps,
        tc.tile_pool(name="xp", bufs=2) as xp,
        tc.tile_pool(name="zps", bufs=3, space="PSUM") as zps,
        tc.tile_pool(name="qps", bufs=3, space="PSUM") as qps,
        tc.tile_pool(name="stp", bufs=2) as stp,
        tc.tile_pool(name="qktp", bufs=2) as qktp,
    ):
        # two G banks; a single accumulation group spans all heads per bank
        # (only the globally-first matmul into each bank carries start=True)
        g1 = gps.tile([48, HEADS * 96], F32, tag="g1")
        g2 = gps.tile([48, HEADS * 48], F32, tag="g2")
        for c in range(NCHUNK):
            zs = zt[c % 2]
            r0 = max(0, R * c - 1)
            r1 = min(H, R * c + R + 1)
            nrows = r1 - r0
            brow0 = r0 - (R * c - 1)  # buf row of image row r0

            xt0 = xp.tile([128, nrows, W], BF16, tag="x0")
            xt1 = xp.tile([64, nrows, W], BF16, tag="x1")
            nc.sync.dma_start(
                xt0[:], x_d[0:128, r0 * W : r1 * W].rearrange("p (r w) -> p r w", w=W)
            )
            nc.sync.dma_start(
                xt1[:], x_d[128:192, r0 * W : r1 * W].rearrange("p (r w) -> p r w", w=W)
            )

            # conv1 into padded z tiles (groups of <=4 rows)
            for g0 in range(0, nrows, 4):
                gn = min(4, nrows - g0)
                for b, (c0, c1) in enumerate(BLKS):
                    p = c1 - c0
                    ps = zps.tile([128, 512], F32, tag="zps")
                    nc.tensor.matmul(
                        ps[0:p, 0 : gn * W],
                        w1t0[:, c0:c1],
                        xt0[:, g0 : g0 + gn, :],
                        start=True,
                        stop=False,
                    )
                    nc.tensor.matmul(
                        ps[0:p, 0 : gn * W],
                        w1t1[:, c0:c1],
                        xt1[:, g0 : g0 + gn, :],
                        start=False,
                        stop=True,
                    )
                    copy(zs[b][:, brow0 + g0 : brow0 + g0 + gn, 1 : 1 + W], ps[0:p, 0 : gn * W])

            if c == NCHUNK - 1:
                # bottom halo row never written this chunk; clear stale data
                for b in range(5):
                    nc.gpsimd.memset(zs[b][:, ZROWS - 1 : ZROWS, :], 0.0)

            # taps: 9 accumulating diagonal matmuls -> qkv rows Rc..Rc+R
            st = [stp.tile([128, R // 4, 4 * W], BF16, tag=f"st{i}", name=f"st{i}") for i in range(3)]
            for g in range(R // 4):
                orow = 1 + 4 * g  # buf row of first output row in this group
                for b, (c0, c1) in enumerate(BLKS):
                    p = c1 - c0
                    ps = qps.tile([128, 512], F32, tag="qps")
                    for t, (di, dj) in enumerate(TAPS):
                        nc.tensor.matmul(
                            ps[0:p, :],
                            diagw[b][t][:],
                            zs[b][:, orow + di : orow + di + 4, 1 + dj : 1 + dj + W],
                            start=(t == 0),
                            stop=(t == 8),
                        )
                    if b < 3:
                        copy(st[b][:, g, :], ps[:, :])
                    elif b == 3:
                        copy(v0[:, c * R * W + g * 512 : c * R * W + (g + 1) * 512], ps[:, :])
                    else:
                        copy(v1[:, c * R * W + g * 512 : c * R * W + (g + 1) * 512], ps[0:64, :])

            # transpose q,k: qkt[:, lt, 0, :] = k^T, [:, lt, 1, :] = q^T
            # batched 3D-out form: out[p, lt, c] = in[c, lt*128 + p]
            st_flat = [s.rearrange("p a b -> p (a b)") for s in st]
            qkt = qktp.tile([128, R, 2, 192], BF16, tag="qkt")
            nc.sync.dma_start_transpose(qkt[:, :, 1, 0:128], st_flat[0][:, :])
            nc.scalar.dma_start_transpose(qkt[:, :, 1, 128:192], st_flat[1][0:64, :])
            nc.sync.dma_start_transpose(qkt[:, :, 0, 0:64], st_flat[1][64:128, :])
            nc.scalar.dma_start_transpose(qkt[:, :, 0, 64:192], st_flat[2][:, :])

            # gram accumulation
            for lt in range(R):
                first = c == 0 and lt == 0
                last = c == NCHUNK - 1 and lt == R - 1
                for h in range(HEADS):
                    nc.tensor.matmul(
                        g1[:, h * 96 : h * 96 + 96],
                        qkt[:, lt, 1, h * DH : (h + 1) * DH],
                        qkt[:, lt, :, h * DH : (h + 1) * DH],
                        start=(first and h == 0),
                        stop=(last and h == HEADS - 1),
                        skip_group_check=True,
                    )
                    nc.tensor.matmul(
                        g2[:, h * DH : (h + 1) * DH],
                        qkt[:, lt, 0, h * DH : (h + 1) * DH],
                        qkt[:, lt, 0, h * DH : (h + 1) * DH],
                        start=(first and h == 0),
                        stop=(last and h == HEADS - 1),
                        skip_group_check=True,
                    )

        nc.vector.tensor_copy(ghs[:, 0 : HEADS * 96], g1[:])
        nc.vector.tensor_copy(ghs[:, HEADS * 96 :], g2[:])

    # ---- phase B ----
    with (
        tc.tile_pool(name="bsb", bufs=1) as bsb,
        tc.tile_pool(name="bps", bufs=1, space="PSUM") as bps,
        tc.tile_pool(name="ops", bufs=2, space="PSUM") as ops,
        tc.tile_pool(name="osb", bufs=4) as osb,
    ):
        attn_bf = bsb.tile([48, HEADS * 48], BF16, tag="attnbf")
        scr = bsb.tile([48, 48], F32, tag="scr")
        scr2 = bsb.tile([48, 48], F32, tag="scr2")
        colv = bsb.tile([48, 1], F32, tag="colv")
        rowv = bsb.tile([1, 48], F32, tag="rowv")
        rkrep = bsb.tile([48, 48], F32, tag="rkrep")
        logits = bsb.tile([48, 48], F32, tag="logits")

        for h in range(HEADS):
            gqk = ghs[:, h * 96 : h * 96 + 48]
            gqq = ghs[:, h * 96 + 48 : h * 96 + 96]
            gkk = ghs[:, HEADS * 96 + h * DH : HEADS * 96 + (h + 1) * DH]

            # rq_inv = 1/max(sqrt(diag(Gqq)),eps), with 1/sqrt(DH) folded in
            nc.vector.tensor_mul(scr[:], gqq, ident48[:])
            nc.vector.reduce_sum(colv[:], scr[:], axis=mybir.AxisListType.X)
            nc.scalar.activation(colv[:], colv[:], AF.Sqrt)
            nc.vector.tensor_scalar_max(colv[:], colv[:], 1e-12)
            nc.vector.reciprocal(colv[:], colv[:])
            nc.vector.tensor_scalar(
                logits[:],
                gqk,
                colv[:],
                1.0 / math.sqrt(DH),
                op0=mybir.AluOpType.mult,
                op1=mybir.AluOpType.mult,
            )

            # rk_inv broadcast along the free (key) dim via diag-as-row
            nc.vector.tensor_mul(scr2[:], gkk, ident48[:])
            ps_row = bps.tile([1, 48], F32, tag="pssmall")
            nc.tensor.matmul(ps_row[:], ones48[:], scr2[:], start=True, stop=True)
            nc.vector.tensor_copy(rowv[:], ps_row[:])
            nc.scalar.activation(rowv[:], rowv[:], AF.Sqrt)
            nc.vector.tensor_scalar_max(rowv[:], rowv[:], 1e-12)
            nc.vector.reciprocal(rowv[:], rowv[:])
            ps_rep = bps.tile([48, 48], F32, tag="pssmall")
            nc.tensor.matmul(ps_rep[:], ones1x48[:], rowv[:], start=True, stop=True)
            nc.vector.tensor_copy(rkrep[:], ps_rep[:])
            nc.vector.tensor_mul(logits[:], logits[:], rkrep[:])

            # softmax over the free (key) dim
            nc.vector.reduce_max(colv[:], logits[:], axis=mybir.AxisListType.X)
            nc.vector.tensor_scalar_sub(logits[:], logits[:], colv[:])
            nc.scalar.activation(logits[:], logits[:], AF.Exp)
            nc.vector.reduce_sum(colv[:], logits[:], axis=mybir.AxisListType.X)
            nc.vector.reciprocal(colv[:], colv[:])
            nc.vector.tensor_scalar_mul(logits[:], logits[:], colv[:])
            nc.vector.tensor_copy(attn_bf[:, h * 48 : (h + 1) * 48], logits[:])

        # block-diagonal attn (bf16)
        bd0 = bsb.tile([128, C], BF16, tag="bd0")
        bd1 = bsb.tile([64, C], BF16, tag="bd1")
        nc.gpsimd.memset(bd0[:], 0.0)
        nc.gpsimd.memset(bd1[:], 0.0)
        nc.sync.dma_start(bd0[0:48, 0:48], attn_bf[:, 0:48])
        nc.sync.dma_start(bd0[48:96, 48:96], attn_bf[:, 48:96])
        nc.sync.dma_start(bd0[96:128, 96:144], attn_bf[0:32, 96:144])
        nc.sync.dma_start(bd1[0:16, 96:144], attn_bf[32:48, 96:144])
        nc.sync.dma_start(bd1[16:64, 144:192], attn_bf[:, 144:192])

        # W_effT = BD(attn).T @ W_outT   [192 x 192], bf16
        weff0 = bsb.tile([128, C], BF16, tag="weff0")
        weff1 = bsb.tile([64, C], BF16, tag="weff1")
        for m0, m1, wt in [(0, 128, weff0), (128, 192, weff1)]:
            pw = bps.tile([128, C], F32, tag="pweff")
            nc.tensor.matmul(pw[0 : m1 - m0, :], bd0[:, m0:m1], woutt0_bf[:], start=True, stop=False)
            nc.tensor.matmul(pw[0 : m1 - m0, :], bd1[:, m0:m1], woutt1_bf[:], start=False, stop=True)
            copy(wt[:], pw[0 : m1 - m0, :])

        # y = W_effT.T @ v
        for g in range(L // 512):
            sl = slice(g * 512, (g + 1) * 512)
            for m0, m1 in [(0, 128), (128, 192)]:
                po = ops.tile([128, 512], F32, tag="ops")
                nc.tensor.matmul(po[0 : m1 - m0, :], weff0[:, m0:m1], v0[:, sl], start=True, stop=False)
                nc.tensor.matmul(po[0 : m1 - m0, :], weff1[:, m0:m1], v1[:, sl], start=False, stop=True)
                ot = osb.tile([m1 - m0, 512], F32, tag=f"o{m0}", name=f"o{m0}")
                copy(ot[:], po[0 : m1 - m0, :])
                nc.sync.dma_start(y_d[m0:m1, sl], ot[:])


_NC_CACHE = None


def _get_nc():
    global _NC_CACHE
    if _NC_CACHE is None:
        _NC_CACHE = build_nc()
    return _NC_CACHE


def kernel(x, w_proj1, w_dw, pos_emb, w_out, _trace=False):
    from concourse.bass_utils import run_bass_kernel_spmd

    import ml_dtypes

    x = np.asarray(x, dtype=np.float32).astype(ml_dtypes.bfloat16)
    w1t = np.ascontiguousarray(
        np.asarray(w_proj1, np.float32).reshape(CQKV, C).T.astype(ml_dtypes.bfloat16)
    )
    wdw = np.ascontiguousarray(np.asarray(w_dw, np.float32).reshape(CQKV, 9))
    woutt = np.ascontiguousarray(np.asarray(w_out, np.float32).reshape(C, C).T)
    # pos_emb adds a per-head constant to every logit in its softmax row;
    # softmax is shift-invariant, so it has no effect on the output.

    nc = _get_nc()
    in_maps = [
        {"x": np.ascontiguousarray(x[b].reshape(C, L)), "w1t": w1t, "wdw": wdw, "woutt": woutt}
        for b in range(N_CORES)
    ]
    res = run_bass_kernel_spmd(nc, in_maps, list(range(N_CORES)), trace=_trace)
    out = np.stack([res.results[b]["y"].reshape(C, H, W) for b in range(N_CORES)])
    if _trace:
        kernel.last_exec_time_ns = res.exec_time_ns
        kernel.last_profile = res
    return out.astype(np.float32)


# revision 8
# speedup vs baseline: 2.2123x; 1.0874x over previous
"""ChannelSA Trainium2 kernel: 8-way batch-parallel across NeuronCores.

kernel(**inputs) takes the FULL inputs (x [8,192,128,128], conv weights,
pos_emb) and returns the FULL output [8,192,128,128] fp32. Each core runs
an identical single-batch program (SPMD, no collectives).

Per-core pipeline:
  z   = W1 @ x                   1x1 conv, fp32r matmuls (K=192 -> 128+64)
  qkv = DW3x3(z)                 9 accumulating diagonal matmuls on TensorE
                                 over a zero-padded bf16 z layout (shifted APs)
  q,k -> bf16 DMA-transpose ->   per-head Gram banks [Gqk|Gqq|Gkk] in PSUM
  logits = Gqk / (|q||k| sqrt(48))   norms taken from the Gram diagonals;
                                 pos_emb is constant per softmax row: a no-op
  attn = softmax(logits)
  y = (W_out @ blockdiag(attn)) @ v   output projection fused with attn@v
"""
import math
from contextlib import ExitStack

import numpy as np

import concourse.bass as bass
import concourse.mybir as mybir
import concourse.tile as tile
from concourse.masks import make_identity

F32 = mybir.dt.float32
F32R = mybir.dt.float32r
BF16 = mybir.dt.bfloat16
AF = mybir.ActivationFunctionType

C = 192
CQKV = 576
H = 128
W = 128
L = H * W
HEADS = 4
DH = 48
R = 8                    # output image rows per chunk
NCHUNK = H // R
PADW = W + 2             # padded row stride in z tiles
ZROWS = R + 2            # rows held per z chunk (1 halo each side)
TAPS = [(di, dj) for di in (-1, 0, 1) for dj in (-1, 0, 1)]
BLKS = [(0, 128), (128, 256), (256, 384), (384, 512), (512, 576)]
N_CORES = 8

_MAX_DRAIN_WAITS = 1


def _patch_tail_drain():
    """The walrus in this image rejects >1 semaphore wait on the Tile tail
    drain instruction; split the waits across a chain of SP nops."""
    if getattr(tile.TileContext, "_drain_patched", False):
        return

    def _drain_and_barrier(self, tick_clock, wait_clock):
        from concourse.vector_clock import ScopedClock

        nc = self.nc
        drain_inst = nc.sync.drain()
        wait_clock.add_sem_waits(
            drain_inst.ins, ScopedClock({None: tick_clock.global_clock})
        )
        si = drain_inst.ins.sync_info
        waits = list(si.on_wait or [])
        if len(waits) > _MAX_DRAIN_WAITS:
            si.on_wait = waits[:_MAX_DRAIN_WAITS]
            rest = waits[_MAX_DRAIN_WAITS:]
            for i in range(0, len(rest), _MAX_DRAIN_WAITS):
                nop = nc.sync.nop(nofuse=True)
                nop.ins.sync_info = mybir.SyncInfo(
                    on_wait=rest[i : i + _MAX_DRAIN_WAITS], on_update=[]
                )
        nc.all_engine_barrier()
        assert self.sems is not None
        popped = nc._tile_sem_poison_stack.pop()
        assert popped is self._sem_poison
        nc.clear_and_free_semaphores(list(self.sems.allocated().values()))
        nc.all_engine_barrier()

    tile.TileContext._drain_and_barrier = _drain_and_barrier
    tile.TileContext._drain_patched = True


def build_nc(split_waits=True):
    _patch_tail_drain()
    nc = bass.Bass("TRN2", target_bir_lowering=False, debug=False)

    x_d = nc.declare_dram_parameter("x", [C, L], BF16, isOutput=False)
    w1t_d = nc.declare_dram_parameter("w1t", [C, CQKV], BF16, isOutput=False)
    wdw_d = nc.declare_dram_parameter("wdw", [CQKV, 9], F32, isOutput=False)
    woutt_d = nc.declare_dram_parameter("woutt", [C, C], F32, isOutput=False)
    y_d = nc.declare_dram_parameter("y", [C, L], F32, isOutput=True)

    with tile.TileContext(nc) as tc, ExitStack() as ctx:
        _body(ctx, tc, x_d, w1t_d, wdw_d, woutt_d, y_d)
    if split_waits:
        # CoreSim can't run the split module (its race detector wants sem
        # updates on every inst); the split is only needed for walrus.
        _split_excess_waits(nc)
    return nc


def _split_excess_waits(nc, maxw=1):
    """This walrus build accepts only one semaphore wait per instruction.
    Move excess waits onto same-engine no-ops inserted just before the
    offending instruction (same-engine program order preserves semantics)."""
    uid = [0]
    for f in nc.m.functions:
        for bb in f.blocks:
            il = bb.instructions
            out = []
            changed = False
            for inst in il:
                si = inst.sync_info
                waits = list(si.on_wait) if si and si.on_wait else []
                if len(waits) > maxw:
                    changed = True
                    rest, keep = waits[:-maxw], waits[-maxw:]
                    for i in range(0, len(rest), maxw):
                        uid[0] += 1
                        out.append(
                            mybir.InstNoOp(
                                name=f"I-waitsplit-{uid[0]}",
                                engine=inst.engine,
                                ins=[],
                                outs=[],
                                sync_info=mybir.SyncInfo(
                                    on_wait=rest[i : i + maxw], on_update=[]
                                ),
                            )
                        )
                    si.on_wait = keep
                out.append(inst)
            if changed:
                bb.instructions = out


def _body(ctx, tc, x_d, w1t_d, wdw_d, woutt_d, y_d):
    nc = tc.nc
    ncopy = [0]

    def copy(dst, src):
        # alternate PSUM->SBUF copies between ACT and DVE
        if ncopy[0] % 2 == 0:
            nc.scalar.copy(dst, src)
        else:
            nc.vector.tensor_copy(dst, src)
        ncopy[0] += 1

    const = ctx.enter_context(tc.tile_pool(name="const", bufs=1))
    persist = ctx.enter_context(tc.tile_pool(name="persist", bufs=1))

    # ---- constants / weights ----
    # K- and M-padded conv1 weights: rows 64:128 of w1t1 and cols 576:640
    # of both are zero so every conv1 matmul is a full 128x128 pass
    w1t0 = const.tile([128, 640], BF16, tag="w1t0")
    w1t1 = const.tile([128, 640], BF16, tag="w1t1")
    nc.gpsimd.memset(w1t0[:], 0.0)
    nc.gpsimd.memset(w1t1[:], 0.0)
    nc.sync.dma_start(w1t0[:, 0:CQKV], w1t_d[0:128, :])
    nc.sync.dma_start(w1t1[0:64, 0:CQKV], w1t_d[128:192, :])

    woutt0 = const.tile([128, C], F32, tag="woutt0")
    woutt1 = const.tile([64, C], F32, tag="woutt1")
    nc.sync.dma_start(woutt0[:], woutt_d[0:128, :])
    nc.sync.dma_start(woutt1[:], woutt_d[128:192, :])
    woutt0_bf = const.tile([128, C], BF16, tag="woutt0bf")
    woutt1_bf = const.tile([64, C], BF16, tag="woutt1bf")
    nc.vector.tensor_copy(woutt0_bf[:], woutt0[:])
    nc.vector.tensor_copy(woutt1_bf[:], woutt1[:])

    ident_bf = const.tile([128, 128], BF16, tag="identbf")
    make_identity(nc, ident_bf[:])
    ident48 = const.tile([48, 48], F32, tag="ident48")
    make_identity(nc, ident48[:])
    ones48 = const.tile([48, 1], F32, tag="ones48")
    nc.gpsimd.memset(ones48[:], 1.0)
    ones1x48 = const.tile([1, 48], F32, tag="ones1x48")
    nc.gpsimd.memset(ones1x48[:], 1.0)

    # dw weights -> 45 diagonal bf16 matrices
    diagw = []
    for b, (c0, c1) in enumerate(BLKS):
        p = c1 - c0
        wdw_sb = const.tile([p, 9], F32, tag=f"wdw{b}")
        nc.sync.dma_start(wdw_sb[:], wdw_d[c0:c1, :])
        row = []
        for t in range(9):
            dt_ = const.tile([p, p], BF16, tag=f"diag{b}_{t}")
            nc.vector.tensor_scalar_mul(dt_[:], ident_bf[0:p, 0:p], wdw_sb[:, t : t + 1])
            row.append(dt_)
        diagw.append(row)

    # ---- persistent state ----
    v0 = persist.tile([128, L], BF16, tag="v0")
    v1 = persist.tile([64, L], BF16, tag="v1")
    zt = [
        [
            persist.tile([c1 - c0, ZROWS, PADW], BF16, tag=f"z{s}_{b}", name=f"z{s}_{b}")
            for b, (c0, c1) in enumerate(BLKS)
        ]
        for s in range(2)
    ]
    for s in range(2):
        for b in range(5):
            nc.gpsimd.memset(zt[s][b][:], 0.0)

    ghs = persist.tile([48, HEADS * 144], F32, tag="ghs")
    xt1_pp = [persist.tile([128, ZROWS, W], BF16, tag=f"xt1_{s}", name=f"xt1_{s}") for s in range(2)]
    for s in range(2):
        nc.gpsimd.memset(xt1_pp[s][:], 0.0)

    # ---- phase A: chunked pipeline ----
    with (
        tc.tile_pool(name="gps", bufs=1, space="PSUM") as gps,
        tc.tile_pool(name="xp", bufs=2) as xp,
        tc.tile_pool(name="zps", bufs=3, space="PSUM") as zps,
        tc.tile_pool(name="qps", bufs=3, space="PSUM") as qps,
        tc.tile_pool(name="stp", bufs=2) as stp,
        tc.tile_pool(name="qktp", bufs=2) as qktp,
    ):
        # two G banks; a single accumulation group spans all heads per bank
        # (only the globally-first matmul into each bank carries start=True)
        g1 = gps.tile([48, HEADS * 96], F32, tag="g1")
        g2 = gps.tile([48, HEADS * 48], F32, tag="g2")
        for c in range(NCHUNK):
            zs = zt[c % 2]
            r0 = max(0, R * c - 1)
            r1 = min(H, R * c + R + 1)
            nrows = r1 - r0
            brow0 = r0 - (R * c - 1)  # buf row of image row r0

            xt0 = xp.tile([128, nrows, W], BF16, tag="x0")
            xt1 = xt1_pp[c % 2]
            nc.sync.dma_start(
                xt0[:], x_d[0:128, r0 * W : r1 * W].rearrange("p (r w) -> p r w", w=W)
            )
            nc.sync.dma_start(
                xt1[0:64, 0:nrows, :],
                x_d[128:192, r0 * W : r1 * W].rearrange("p (r w) -> p r w", w=W),
            )

            # conv1 into padded z tiles (groups of <=4 rows)
            for g0 in range(0, nrows, 4):
                gn = min(4, nrows - g0)
                for b, (c0, c1) in enumerate(BLKS):
                    p = c1 - c0
                    ps = zps.tile([128, 512], F32, tag="zps")
                    nc.tensor.matmul(
                        ps[:, 0 : gn * W],
                        w1t0[:, c0 : c0 + 128],
                        xt0[:, g0 : g0 + gn, :],
                        start=True,
                        stop=False,
                    )
                    nc.tensor.matmul(
                        ps[:, 0 : gn * W],
                        w1t1[:, c0 : c0 + 128],
                        xt1[:, g0 : g0 + gn, :],
                        start=False,
                        stop=True,
                    )
                    copy(zs[b][:, brow0 + g0 : brow0 + g0 + gn, 1 : 1 + W], ps[0:p, 0 : gn * W])

            if c == NCHUNK - 1:
                # bottom halo row never written this chunk; clear stale data
                for b in range(5):
                    nc.gpsimd.memset(zs[b][:, ZROWS - 1 : ZROWS, :], 0.0)

            # taps: 9 accumulating diagonal matmuls -> qkv rows Rc..Rc+R
            st = [stp.tile([128, R // 4, 4 * W], BF16, tag=f"st{i}", name=f"st{i}") for i in range(3)]
            for g in range(R // 4):
                orow = 1 + 4 * g  # buf row of first output row in this group
                for b, (c0, c1) in enumerate(BLKS):
                    p = c1 - c0
                    ps = qps.tile([128, 512], F32, tag="qps")
                    for t, (di, dj) in enumerate(TAPS):
                        nc.tensor.matmul(
                            ps[0:p, :],
                            diagw[b][t][:],
                            zs[b][:, orow + di : orow + di + 4, 1 + dj : 1 + dj + W],
                            start=(t == 0),
                            stop=(t == 8),
                        )
                    if b < 3:
                        copy(st[b][:, g, :], ps[:, :])
                    elif b == 3:
                        copy(v0[:, c * R * W + g * 512 : c * R * W + (g + 1) * 512], ps[:, :])
                    else:
                        copy(v1[:, c * R * W + g * 512 : c * R * W + (g + 1) * 512], ps[0:64, :])

            # transpose q,k: qkt[:, lt, 0, :] = k^T, [:, lt, 1, :] = q^T
            # batched 3D-out form: out[p, lt, c] = in[c, lt*128 + p]
            st_flat = [s.rearrange("p a b -> p (a b)") for s in st]
            qkt = qktp.tile([128, R, 2, 192], BF16, tag="qkt")
            nc.sync.dma_start_transpose(qkt[:, :, 1, 0:128], st_flat[0][:, :])
            nc.scalar.dma_start_transpose(qkt[:, :, 1, 128:192], st_flat[1][0:64, :])
            nc.sync.dma_start_transpose(qkt[:, :, 0, 0:64], st_flat[1][64:128, :])
            nc.scalar.dma_start_transpose(qkt[:, :, 0, 64:192], st_flat[2][:, :])

            # gram accumulation
            for lt in range(R):
                first = c == 0 and lt == 0
                last = c == NCHUNK - 1 and lt == R - 1
                for h in range(HEADS):
                    nc.tensor.matmul(
                        g1[:, h * 96 : h * 96 + 96],
                        qkt[:, lt, 1, h * DH : (h + 1) * DH],
                        qkt[:, lt, :, h * DH : (h + 1) * DH],
                        start=(first and h == 0),
                        stop=(last and h == HEADS - 1),
                        skip_group_check=True,
                    )
                    nc.tensor.matmul(
                        g2[:, h * DH : (h + 1) * DH],
                        qkt[:, lt, 0, h * DH : (h + 1) * DH],
                        qkt[:, lt, 0, h * DH : (h + 1) * DH],
                        start=(first and h == 0),
                        stop=(last and h == HEADS - 1),
                        skip_group_check=True,
                    )

        nc.vector.tensor_copy(ghs[:, 0 : HEADS * 96], g1[:])
        nc.vector.tensor_copy(ghs[:, HEADS * 96 :], g2[:])

    # ---- phase B ----
    with (
        tc.tile_pool(name="bsb", bufs=1) as bsb,
        tc.tile_pool(name="bps", bufs=1, space="PSUM") as bps,
        tc.tile_pool(name="ops", bufs=4, space="PSUM") as ops,
        tc.tile_pool(name="osb", bufs=4) as osb,
    ):
        attn_bf = bsb.tile([48, HEADS * 48], BF16, tag="attnbf")
        scr = bsb.tile([48, 48], F32, tag="scr")
        scr2 = bsb.tile([48, 48], F32, tag="scr2")
        colv = bsb.tile([48, 1], F32, tag="colv")
        rowv = bsb.tile([1, 48], F32, tag="rowv")
        rkrep = bsb.tile([48, 48], F32, tag="rkrep")
        logits = bsb.tile([48, 48], F32, tag="logits")

        for h in range(HEADS):
            gqk = ghs[:, h * 96 : h * 96 + 48]
            gqq = ghs[:, h * 96 + 48 : h * 96 + 96]
            gkk = ghs[:, HEADS * 96 + h * DH : HEADS * 96 + (h + 1) * DH]

            # rq_inv = 1/max(sqrt(diag(Gqq)),eps), with 1/sqrt(DH) folded in
            nc.vector.tensor_mul(scr[:], gqq, ident48[:])
            nc.vector.reduce_sum(colv[:], scr[:], axis=mybir.AxisListType.X)
            nc.scalar.activation(colv[:], colv[:], AF.Sqrt)
            nc.vector.tensor_scalar_max(colv[:], colv[:], 1e-12)
            nc.vector.reciprocal(colv[:], colv[:])
            nc.vector.tensor_scalar(
                logits[:],
                gqk,
                colv[:],
                1.0 / math.sqrt(DH),
                op0=mybir.AluOpType.mult,
                op1=mybir.AluOpType.mult,
            )

            # rk_inv broadcast along the free (key) dim via diag-as-row
            nc.vector.tensor_mul(scr2[:], gkk, ident48[:])
            ps_row = bps.tile([1, 48], F32, tag="pssmall")
            nc.tensor.matmul(ps_row[:], ones48[:], scr2[:], start=True, stop=True)
            nc.vector.tensor_copy(rowv[:], ps_row[:])
            nc.scalar.activation(rowv[:], rowv[:], AF.Sqrt)
            nc.vector.tensor_scalar_max(rowv[:], rowv[:], 1e-12)
            nc.vector.reciprocal(rowv[:], rowv[:])
            ps_rep = bps.tile([48, 48], F32, tag="pssmall")
            nc.tensor.matmul(ps_rep[:], ones1x48[:], rowv[:], start=True, stop=True)
            nc.vector.tensor_copy(rkrep[:], ps_rep[:])
            nc.vector.tensor_mul(logits[:], logits[:], rkrep[:])

            # softmax over the free (key) dim
            nc.vector.reduce_max(colv[:], logits[:], axis=mybir.AxisListType.X)
            nc.vector.tensor_scalar_sub(logits[:], logits[:], colv[:])
            nc.scalar.activation(logits[:], logits[:], AF.Exp)
            nc.vector.reduce_sum(colv[:], logits[:], axis=mybir.AxisListType.X)
            nc.vector.reciprocal(colv[:], colv[:])
            nc.vector.tensor_scalar_mul(logits[:], logits[:], colv[:])
            nc.vector.tensor_copy(attn_bf[:, h * 48 : (h + 1) * 48], logits[:])

        # block-diagonal attn (bf16)
        bd0 = bsb.tile([128, C], BF16, tag="bd0")
        bd1 = bsb.tile([64, C], BF16, tag="bd1")
        nc.gpsimd.memset(bd0[:], 0.0)
        nc.gpsimd.memset(bd1[:], 0.0)
        nc.sync.dma_start(bd0[0:48, 0:48], attn_bf[:, 0:48])
        nc.sync.dma_start(bd0[48:96, 48:96], attn_bf[:, 48:96])
        nc.sync.dma_start(bd0[96:128, 96:144], attn_bf[0:32, 96:144])
        nc.sync.dma_start(bd1[0:16, 96:144], attn_bf[32:48, 96:144])
        nc.sync.dma_start(bd1[16:64, 144:192], attn_bf[:, 144:192])

        # W_effT = BD(attn).T @ W_outT   [192 x 192], bf16
        weff0 = bsb.tile([128, C], BF16, tag="weff0")
        weff1 = bsb.tile([64, C], BF16, tag="weff1")
        for m0, m1, wt in [(0, 128, weff0), (128, 192, weff1)]:
            pw = bps.tile([128, C], F32, tag="pweff")
            nc.tensor.matmul(pw[0 : m1 - m0, :], bd0[:, m0:m1], woutt0_bf[:], start=True, stop=False)
            nc.tensor.matmul(pw[0 : m1 - m0, :], bd1[:, m0:m1], woutt1_bf[:], start=False, stop=True)
            copy(wt[:], pw[0 : m1 - m0, :])

        # y = W_effT.T @ v
        for g in range(L // 512):
            sl = slice(g * 512, (g + 1) * 512)
            for m0, m1 in [(0, 128), (128, 192)]:
                po = ops.tile([128, 512], F32, tag="ops")
                nc.tensor.matmul(po[0 : m1 - m0, :], weff0[:, m0:m1], v0[:, sl], start=True, stop=False)
                nc.tensor.matmul(po[0 : m1 - m0, :], weff1[:, m0:m1], v1[:, sl], start=False, stop=True)
                ot = osb.tile([m1 - m0, 512], F32, tag=f"o{m0}", name=f"o{m0}")
                copy(ot[:], po[0 : m1 - m0, :])
                nc.sync.dma_start(y_d[m0:m1, sl], ot[:])


_NC_CACHE = None


def _get_nc():
    global _NC_CACHE
    if _NC_CACHE is None:
        _NC_CACHE = build_nc()
    return _NC_CACHE


def kernel(x, w_proj1, w_dw, pos_emb, w_out, _trace=False):
    from concourse.bass_utils import run_bass_kernel_spmd

    import ml_dtypes

    x = np.asarray(x, dtype=np.float32).astype(ml_dtypes.bfloat16)
    w1t = np.ascontiguousarray(
        np.asarray(w_proj1, np.float32).reshape(CQKV, C).T.astype(ml_dtypes.bfloat16)
    )
    wdw = np.ascontiguousarray(np.asarray(w_dw, np.float32).reshape(CQKV, 9))
    woutt = np.ascontiguousarray(np.asarray(w_out, np.float32).reshape(C, C).T)
    # pos_emb adds a per-head constant to every logit in its softmax row;
    # softmax is shift-invariant, so it has no effect on the output.

    nc = _get_nc()
    in_maps = [
        {"x": np.ascontiguousarray(x[b].reshape(C, L)), "w1t": w1t, "wdw": wdw, "woutt": woutt}
        for b in range(N_CORES)
    ]
    res = run_bass_kernel_spmd(nc, in_maps, list(range(N_CORES)), trace=_trace)
    out = np.stack([res.results[b]["y"].reshape(C, H, W) for b in range(N_CORES)])
    if _trace:
        kernel.last_exec_time_ns = res.exec_time_ns
        kernel.last_profile = res
    return out.astype(np.float32)


# revision 9
# speedup vs baseline: 2.2506x; 1.0173x over previous
"""ChannelSA Trainium2 kernel: 8-way batch-parallel across NeuronCores.

kernel(**inputs) takes the FULL inputs (x [8,192,128,128], conv weights,
pos_emb) and returns the FULL output [8,192,128,128] fp32. Each core runs
an identical single-batch program (SPMD, no collectives).

Per-core pipeline:
  z   = W1 @ x                   1x1 conv, fp32r matmuls (K=192 -> 128+64)
  qkv = DW3x3(z)                 9 accumulating diagonal matmuls on TensorE
                                 over a zero-padded bf16 z layout (shifted APs)
  q,k -> bf16 DMA-transpose ->   per-head Gram banks [Gqk|Gqq|Gkk] in PSUM
  logits = Gqk / (|q||k| sqrt(48))   norms taken from the Gram diagonals;
                                 pos_emb is constant per softmax row: a no-op
  attn = softmax(logits)
  y = (W_out @ blockdiag(attn)) @ v   output projection fused with attn@v
"""
import math
from contextlib import ExitStack

import numpy as np

import concourse.bass as bass
import concourse.mybir as mybir
import concourse.tile as tile
from concourse.masks import make_identity

F32 = mybir.dt.float32
F32R = mybir.dt.float32r
BF16 = mybir.dt.bfloat16
AF = mybir.ActivationFunctionType

C = 192
CQKV = 576
H = 128
W = 128
L = H * W
HEADS = 4
DH = 48
R = 8                    # output image rows per chunk
NCHUNK = H // R
PADW = W + 2             # padded row stride in z tiles
ZROWS = R + 2            # rows held per z chunk (1 halo each side)
TAPS = [(di, dj) for di in (-1, 0, 1) for dj in (-1, 0, 1)]
BLKS = [(0, 128), (128, 256), (256, 384), (384, 512), (512, 576)]
N_CORES = 8

_MAX_DRAIN_WAITS = 1


def _patch_tail_drain():
    """The walrus in this image rejects >1 semaphore wait on the Tile tail
    drain instruction; split the waits across a chain of SP nops."""
    if getattr(tile.TileContext, "_drain_patched", False):
        return

    def _drain_and_barrier(self, tick_clock, wait_clock):
        from concourse.vector_clock import ScopedClock

        nc = self.nc
        drain_inst = nc.sync.drain()
        wait_clock.add_sem_waits(
            drain_inst.ins, ScopedClock({None: tick_clock.global_clock})
        )
        si = drain_inst.ins.sync_info
        waits = list(si.on_wait or [])
        if len(waits) > _MAX_DRAIN_WAITS:
            si.on_wait = waits[:_MAX_DRAIN_WAITS]
            rest = waits[_MAX_DRAIN_WAITS:]
            for i in range(0, len(rest), _MAX_DRAIN_WAITS):
                nop = nc.sync.nop(nofuse=True)
                nop.ins.sync_info = mybir.SyncInfo(
                    on_wait=rest[i : i + _MAX_DRAIN_WAITS], on_update=[]
                )
        nc.all_engine_barrier()
        assert self.sems is not None
        popped = nc._tile_sem_poison_stack.pop()
        assert popped is self._sem_poison
        nc.clear_and_free_semaphores(list(self.sems.allocated().values()))
        nc.all_engine_barrier()

    tile.TileContext._drain_and_barrier = _drain_and_barrier
    tile.TileContext._drain_patched = True


def build_nc(split_waits=True):
    _patch_tail_drain()
    nc = bass.Bass("TRN2", target_bir_lowering=False, debug=False)

    x_d = nc.declare_dram_parameter("x", [C, L], BF16, isOutput=False)
    w1t_d = nc.declare_dram_parameter("w1t", [C, CQKV], BF16, isOutput=False)
    wdw_d = nc.declare_dram_parameter("wdw", [CQKV, 9], F32, isOutput=False)
    woutt_d = nc.declare_dram_parameter("woutt", [C, C], F32, isOutput=False)
    y_d = nc.declare_dram_parameter("y", [C, L], F32, isOutput=True)

    with tile.TileContext(nc) as tc, ExitStack() as ctx:
        _body(ctx, tc, x_d, w1t_d, wdw_d, woutt_d, y_d)
    if split_waits:
        # CoreSim can't run the split module (its race detector wants sem
        # updates on every inst); the split is only needed for walrus.
        _split_excess_waits(nc)
    return nc


def _split_excess_waits(nc, maxw=1):
    """This walrus build accepts only one semaphore wait per instruction.
    Move excess waits onto same-engine no-ops inserted just before the
    offending instruction (same-engine program order preserves semantics)."""
    uid = [0]
    for f in nc.m.functions:
        for bb in f.blocks:
            il = bb.instructions
            out = []
            changed = False
            for inst in il:
                si = inst.sync_info
                waits = list(si.on_wait) if si and si.on_wait else []
                if len(waits) > maxw:
                    changed = True
                    rest, keep = waits[:-maxw], waits[-maxw:]
                    for i in range(0, len(rest), maxw):
                        uid[0] += 1
                        out.append(
                            mybir.InstNoOp(
                                name=f"I-waitsplit-{uid[0]}",
                                engine=inst.engine,
                                ins=[],
                                outs=[],
                                sync_info=mybir.SyncInfo(
                                    on_wait=rest[i : i + maxw], on_update=[]
                                ),
                            )
                        )
                    si.on_wait = keep
                out.append(inst)
            if changed:
                bb.instructions = out


def _body(ctx, tc, x_d, w1t_d, wdw_d, woutt_d, y_d):
    nc = tc.nc
    ncopy = [0]

    def copy(dst, src):
        # alternate PSUM->SBUF copies between ACT and DVE
        if ncopy[0] % 2 == 0:
            nc.scalar.copy(dst, src)
        else:
            nc.vector.tensor_copy(dst, src)
        ncopy[0] += 1

    const = ctx.enter_context(tc.tile_pool(name="const", bufs=1))
    persist = ctx.enter_context(tc.tile_pool(name="persist", bufs=1))

    # ---- constants / weights ----
    # K- and M-padded conv1 weights: rows 64:128 of w1t1 and cols 576:640
    # of both are zero so every conv1 matmul is a full 128x128 pass
    w1t0 = const.tile([128, 640], BF16, tag="w1t0")
    w1t1 = const.tile([128, 640], BF16, tag="w1t1")
    nc.gpsimd.memset(w1t0[:], 0.0)
    nc.gpsimd.memset(w1t1[:], 0.0)
    nc.sync.dma_start(w1t0[:, 0:CQKV], w1t_d[0:128, :])
    nc.sync.dma_start(w1t1[0:64, 0:CQKV], w1t_d[128:192, :])

    woutt0 = const.tile([128, C], F32, tag="woutt0")
    woutt1 = const.tile([64, C], F32, tag="woutt1")
    nc.sync.dma_start(woutt0[:], woutt_d[0:128, :])
    nc.sync.dma_start(woutt1[:], woutt_d[128:192, :])
    woutt0_bf = const.tile([128, C], BF16, tag="woutt0bf")
    woutt1_bf = const.tile([64, C], BF16, tag="woutt1bf")
    nc.vector.tensor_copy(woutt0_bf[:], woutt0[:])
    nc.vector.tensor_copy(woutt1_bf[:], woutt1[:])

    ident_bf = const.tile([128, 128], BF16, tag="identbf")
    make_identity(nc, ident_bf[:])
    ident48 = const.tile([48, 48], F32, tag="ident48")
    make_identity(nc, ident48[:])
    ones48 = const.tile([48, 1], F32, tag="ones48")
    nc.gpsimd.memset(ones48[:], 1.0)
    ones1x48 = const.tile([1, 48], F32, tag="ones1x48")
    nc.gpsimd.memset(ones1x48[:], 1.0)

    # dw weights -> 45 diagonal bf16 matrices
    diagw = []
    for b, (c0, c1) in enumerate(BLKS):
        p = c1 - c0
        wdw_sb = const.tile([p, 9], F32, tag=f"wdw{b}")
        nc.sync.dma_start(wdw_sb[:], wdw_d[c0:c1, :])
        row = []
        for t in range(9):
            dt_ = const.tile([p, p], BF16, tag=f"diag{b}_{t}")
            nc.vector.tensor_scalar_mul(dt_[:], ident_bf[0:p, 0:p], wdw_sb[:, t : t + 1])
            row.append(dt_)
        diagw.append(row)

    # ---- persistent state ----
    v0 = persist.tile([128, L], BF16, tag="v0")
    v1 = persist.tile([128, L], BF16, tag="v1")
    nc.gpsimd.memset(v1[64:128, :], 0.0)
    zt = [
        [
            persist.tile([c1 - c0, ZROWS, PADW], BF16, tag=f"z{s}_{b}", name=f"z{s}_{b}")
            for b, (c0, c1) in enumerate(BLKS)
        ]
        for s in range(2)
    ]
    for s in range(2):
        for b in range(5):
            nc.gpsimd.memset(zt[s][b][:], 0.0)

    ghs = persist.tile([48, HEADS * 144], F32, tag="ghs")
    xt1_pp = [persist.tile([128, ZROWS, W], BF16, tag=f"xt1_{s}", name=f"xt1_{s}") for s in range(2)]
    for s in range(2):
        nc.gpsimd.memset(xt1_pp[s][:], 0.0)

    # ---- phase A: chunked pipeline ----
    with (
        tc.tile_pool(name="gps", bufs=1, space="PSUM") as gps,
        tc.tile_pool(name="xp", bufs=2) as xp,
        tc.tile_pool(name="zps", bufs=3, space="PSUM") as zps,
        tc.tile_pool(name="qps", bufs=3, space="PSUM") as qps,
        tc.tile_pool(name="stp", bufs=2) as stp,
        tc.tile_pool(name="qktp", bufs=2) as qktp,
    ):
        # two G banks; a single accumulation group spans all heads per bank
        # (only the globally-first matmul into each bank carries start=True)
        g1 = gps.tile([48, HEADS * 96], F32, tag="g1")
        g2 = gps.tile([48, HEADS * 48], F32, tag="g2")
        for c in range(NCHUNK):
            zs = zt[c % 2]
            r0 = max(0, R * c - 1)
            r1 = min(H, R * c + R + 1)
            nrows = r1 - r0
            brow0 = r0 - (R * c - 1)  # buf row of image row r0

            xt0 = xp.tile([128, nrows, W], BF16, tag="x0")
            xt1 = xt1_pp[c % 2]
            nc.sync.dma_start(
                xt0[:], x_d[0:128, r0 * W : r1 * W].rearrange("p (r w) -> p r w", w=W)
            )
            nc.sync.dma_start(
                xt1[0:64, 0:nrows, :],
                x_d[128:192, r0 * W : r1 * W].rearrange("p (r w) -> p r w", w=W),
            )

            # conv1 into padded z tiles (groups of <=4 rows)
            for g0 in range(0, nrows, 4):
                gn = min(4, nrows - g0)
                for b, (c0, c1) in enumerate(BLKS):
                    p = c1 - c0
                    ps = zps.tile([128, 512], F32, tag="zps")
                    nc.tensor.matmul(
                        ps[:, 0 : gn * W],
                        w1t0[:, c0 : c0 + 128],
                        xt0[:, g0 : g0 + gn, :],
                        start=True,
                        stop=False,
                    )
                    nc.tensor.matmul(
                        ps[:, 0 : gn * W],
                        w1t1[:, c0 : c0 + 128],
                        xt1[:, g0 : g0 + gn, :],
                        start=False,
                        stop=True,
                    )
                    copy(zs[b][:, brow0 + g0 : brow0 + g0 + gn, 1 : 1 + W], ps[0:p, 0 : gn * W])

            if c == NCHUNK - 1:
                # bottom halo row never written this chunk; clear stale data
                for b in range(5):
                    nc.gpsimd.memset(zs[b][:, ZROWS - 1 : ZROWS, :], 0.0)

            # taps: 9 accumulating diagonal matmuls -> qkv rows Rc..Rc+R
            st = [stp.tile([128, R // 4, 4 * W], BF16, tag=f"st{i}", name=f"st{i}") for i in range(3)]
            for g in range(R // 4):
                orow = 1 + 4 * g  # buf row of first output row in this group
                for b, (c0, c1) in enumerate(BLKS):
                    p = c1 - c0
                    ps = qps.tile([128, 512], F32, tag="qps")
                    for t, (di, dj) in enumerate(TAPS):
                        nc.tensor.matmul(
                            ps[0:p, :],
                            diagw[b][t][:],
                            zs[b][:, orow + di : orow + di + 4, 1 + dj : 1 + dj + W],
                            start=(t == 0),
                            stop=(t == 8),
                        )
                    if b < 3:
                        copy(st[b][:, g, :], ps[:, :])
                    elif b == 3:
                        copy(v0[:, c * R * W + g * 512 : c * R * W + (g + 1) * 512], ps[:, :])
                    else:
                        copy(v1[0:64, c * R * W + g * 512 : c * R * W + (g + 1) * 512], ps[0:64, :])

            # transpose q,k: qkt[:, lt, 0, :] = k^T, [:, lt, 1, :] = q^T
            # batched 3D-out form: out[p, lt, c] = in[c, lt*128 + p]
            st_flat = [s.rearrange("p a b -> p (a b)") for s in st]
            qkt = qktp.tile([128, R, 2, 192], BF16, tag="qkt")
            nc.sync.dma_start_transpose(qkt[:, :, 1, 0:128], st_flat[0][:, :])
            nc.scalar.dma_start_transpose(qkt[:, :, 1, 128:192], st_flat[1][0:64, :])
            nc.sync.dma_start_transpose(qkt[:, :, 0, 0:64], st_flat[1][64:128, :])
            nc.scalar.dma_start_transpose(qkt[:, :, 0, 64:192], st_flat[2][:, :])

            # gram accumulation
            for lt in range(R):
                first = c == 0 and lt == 0
                last = c == NCHUNK - 1 and lt == R - 1
                for h in range(HEADS):
                    nc.tensor.matmul(
                        g1[:, h * 96 : h * 96 + 96],
                        qkt[:, lt, 1, h * DH : (h + 1) * DH],
                        qkt[:, lt, :, h * DH : (h + 1) * DH],
                        start=(first and h == 0),
                        stop=(last and h == HEADS - 1),
                        skip_group_check=True,
                    )
                    nc.tensor.matmul(
                        g2[:, h * DH : (h + 1) * DH],
                        qkt[:, lt, 0, h * DH : (h + 1) * DH],
                        qkt[:, lt, 0, h * DH : (h + 1) * DH],
                        start=(first and h == 0),
                        stop=(last and h == HEADS - 1),
                        skip_group_check=True,
                    )

        nc.vector.tensor_copy(ghs[:, 0 : HEADS * 96], g1[:])
        nc.vector.tensor_copy(ghs[:, HEADS * 96 :], g2[:])

    # ---- phase B ----
    with (
        tc.tile_pool(name="bsb", bufs=1) as bsb,
        tc.tile_pool(name="bps", bufs=1, space="PSUM") as bps,
        tc.tile_pool(name="ops", bufs=4, space="PSUM") as ops,
        tc.tile_pool(name="osb", bufs=4) as osb,
    ):
        attn_bf = bsb.tile([48, HEADS * 48], BF16, tag="attnbf")
        scr = bsb.tile([48, 48], F32, tag="scr")
        scr2 = bsb.tile([48, 48], F32, tag="scr2")
        colv = bsb.tile([48, 1], F32, tag="colv")
        rowv = bsb.tile([1, 48], F32, tag="rowv")
        rkrep = bsb.tile([48, 48], F32, tag="rkrep")
        logits = bsb.tile([48, 48], F32, tag="logits")

        for h in range(HEADS):
            gqk = ghs[:, h * 96 : h * 96 + 48]
            gqq = ghs[:, h * 96 + 48 : h * 96 + 96]
            gkk = ghs[:, HEADS * 96 + h * DH : HEADS * 96 + (h + 1) * DH]

            # rq_inv = 1/max(sqrt(diag(Gqq)),eps), with 1/sqrt(DH) folded in
            nc.vector.tensor_mul(scr[:], gqq, ident48[:])
            nc.vector.reduce_sum(colv[:], scr[:], axis=mybir.AxisListType.X)
            nc.scalar.activation(colv[:], colv[:], AF.Sqrt)
            nc.vector.tensor_scalar_max(colv[:], colv[:], 1e-12)
            nc.vector.reciprocal(colv[:], colv[:])
            nc.vector.tensor_scalar(
                logits[:],
                gqk,
                colv[:],
                1.0 / math.sqrt(DH),
                op0=mybir.AluOpType.mult,
                op1=mybir.AluOpType.mult,
            )

            # rk_inv broadcast along the free (key) dim via diag-as-row
            nc.vector.tensor_mul(scr2[:], gkk, ident48[:])
            ps_row = bps.tile([1, 48], F32, tag="pssmall")
            nc.tensor.matmul(ps_row[:], ones48[:], scr2[:], start=True, stop=True)
            nc.vector.tensor_copy(rowv[:], ps_row[:])
            nc.scalar.activation(rowv[:], rowv[:], AF.Sqrt)
            nc.vector.tensor_scalar_max(rowv[:], rowv[:], 1e-12)
            nc.vector.reciprocal(rowv[:], rowv[:])
            ps_rep = bps.tile([48, 48], F32, tag="pssmall")
            nc.tensor.matmul(ps_rep[:], ones1x48[:], rowv[:], start=True, stop=True)
            nc.vector.tensor_copy(rkrep[:], ps_rep[:])
            nc.vector.tensor_mul(logits[:], logits[:], rkrep[:])

            # softmax over the free (key) dim
            nc.vector.reduce_max(colv[:], logits[:], axis=mybir.AxisListType.X)
            nc.vector.tensor_scalar_sub(logits[:], logits[:], colv[:])
            nc.scalar.activation(logits[:], logits[:], AF.Exp)
            nc.vector.reduce_sum(colv[:], logits[:], axis=mybir.AxisListType.X)
            nc.vector.reciprocal(colv[:], colv[:])
            nc.vector.tensor_scalar_mul(logits[:], logits[:], colv[:])
            nc.vector.tensor_copy(attn_bf[:, h * 48 : (h + 1) * 48], logits[:])

        # block-diagonal attn (bf16)
        bd0 = bsb.tile([128, C], BF16, tag="bd0")
        bd1 = bsb.tile([64, C], BF16, tag="bd1")
        nc.gpsimd.memset(bd0[:], 0.0)
        nc.gpsimd.memset(bd1[:], 0.0)
        nc.sync.dma_start(bd0[0:48, 0:48], attn_bf[:, 0:48])
        nc.sync.dma_start(bd0[48:96, 48:96], attn_bf[:, 48:96])
        nc.sync.dma_start(bd0[96:128, 96:144], attn_bf[0:32, 96:144])
        nc.sync.dma_start(bd1[0:16, 96:144], attn_bf[32:48, 96:144])
        nc.sync.dma_start(bd1[16:64, 144:192], attn_bf[:, 144:192])

        # W_effT = BD(attn).T @ W_outT   [192 x 192], bf16
        weff0 = bsb.tile([128, 256], BF16, tag="weff0")
        weff1 = bsb.tile([128, 256], BF16, tag="weff1")
        nc.gpsimd.memset(weff0[:], 0.0)
        nc.gpsimd.memset(weff1[:], 0.0)
        for m0, m1, wt in [(0, 128, weff0), (128, 192, weff1)]:
            pw = bps.tile([128, C], F32, tag="pweff")
            nc.tensor.matmul(pw[0 : m1 - m0, :], bd0[:, m0:m1], woutt0_bf[:], start=True, stop=False)
            nc.tensor.matmul(pw[0 : m1 - m0, :], bd1[:, m0:m1], woutt1_bf[:], start=False, stop=True)
            copy(wt[0 : m1 - m0, 0:C], pw[0 : m1 - m0, :])

        # y = W_effT.T @ v
        for g in range(L // 512):
            sl = slice(g * 512, (g + 1) * 512)
            for m0, m1 in [(0, 128), (128, 192)]:
                po = ops.tile([128, 512], F32, tag="ops")
                nc.tensor.matmul(po[:, :], weff0[:, m0 : m0 + 128], v0[:, sl], start=True, stop=False)
                nc.tensor.matmul(po[:, :], weff1[:, m0 : m0 + 128], v1[:, sl], start=False, stop=True)
                ot = osb.tile([m1 - m0, 512], F32, tag=f"o{m0}", name=f"o{m0}")
                copy(ot[:], po[0 : m1 - m0, :])
                nc.sync.dma_start(y_d[m0:m1, sl], ot[:])


_NC_CACHE = None


def _get_nc():
    global _NC_CACHE
    if _NC_CACHE is None:
        _NC_CACHE = build_nc()
    return _NC_CACHE


def kernel(x, w_proj1, w_dw, pos_emb, w_out, _trace=False):
    from concourse.bass_utils import run_bass_kernel_spmd

    import ml_dtypes

    x = np.asarray(x, dtype=np.float32).astype(ml_dtypes.bfloat16)
    w1t = np.ascontiguousarray(
        np.asarray(w_proj1, np.float32).reshape(CQKV, C).T.astype(ml_dtypes.bfloat16)
    )
    wdw = np.ascontiguousarray(np.asarray(w_dw, np.float32).reshape(CQKV, 9))
    woutt = np.ascontiguousarray(np.asarray(w_out, np.float32).reshape(C, C).T)
    # pos_emb adds a per-head constant to every logit in its softmax row;
    # softmax is shift-invariant, so it has no effect on the output.

    nc = _get_nc()
    in_maps = [
        {"x": np.ascontiguousarray(x[b].reshape(C, L)), "w1t": w1t, "wdw": wdw, "woutt": woutt}
        for b in range(N_CORES)
    ]
    res = run_bass_kernel_spmd(nc, in_maps, list(range(N_CORES)), trace=_trace)
    out = np.stack([res.results[b]["y"].reshape(C, H, W) for b in range(N_CORES)])
    if _trace:
        kernel.last_exec_time_ns = res.exec_time_ns
        kernel.last_profile = res
    return out.astype(np.float32)
